# revision 8
# baseline (speedup 1.0000x reference)
"""Trainium2 Bass kernel for masked attention with pre-softmax-score AV matmul.

Reference semantics (faithful to the source module's bug):
    a = (Q @ K^T) / sqrt(D);  a = where(mask, -1e9, a)
    attn_p = softmax(a, axis=-1)
    attn_v = a @ V            # uses pre-softmax masked scores, NOT attn_p
    returns (attn_v, attn_p)

Shapes: Q,K,V (2,8,2048,64) f32; mask (2,8,2048,2048) bool.
Sharding: B*H = 16 head-slices, 2 per core across 8 cores (pure data
parallel, no collectives).
"""

import sys

sys.path.insert(0, "/opt/trn_rl_repo")

import numpy as np

import concourse.bass as bass
import concourse.tile as tile
from concourse import mybir
from concourse.bass_utils import run_bass_kernel_spmd
from concourse.masks import make_identity

B, H, S, D = 2, 8, 2048, 64
N_CORES = 8
HEADS_PER_CORE = (B * H) // N_CORES  # 2
P = 128                              # partition tile (q rows per tile)
NQT = S // P                         # 16 q-tiles per head
KC = S // P                          # 16 k-chunks of 128
GRP = 4                              # q-tiles per AV matmul group
SCALE = 1.0 / np.sqrt(np.float32(D))  # 0.125, exact in fp32
NEG = -1.0e9

f32 = mybir.dt.float32
f32r = mybir.dt.float32r
bf16 = mybir.dt.bfloat16
u8 = mybir.dt.uint8

_COMPILED = {}


def legalize_waits(nc):
    """This walrus build allows only ONE sync-wait command per instruction.

    Tile's wait assigner can attach several (one per upstream engine), which
    fails codegen with 'Too many sync wait commands'. Hoist all but the last
    wait onto preceding same-engine NoOps (program order on the engine's
    sequencer makes this semantically identical).
    """
    n_fixed = 0
    for fn in nc.m.functions:
        for blk in fn.blocks:
            insts = list(blk.instructions)
            new_list = []
            changed = False
            for inst in insts:
                si = inst.sync_info
                if si is not None and si.on_wait is not None and len(si.on_wait) > 1:
                    waits = list(si.on_wait)
                    for wi, w in enumerate(waits[:-1]):
                        new_list.append(
                            mybir.InstNoOp(
                                name=f"{inst.name}-wn{wi}",
                                engine=inst.engine,
                                sync_info=mybir.SyncInfo(on_wait=[w], on_update=[]),
                            )
                        )
                    inst.sync_info = mybir.SyncInfo(
                        on_wait=waits[-1:], on_update=list(si.on_update or [])
                    )
                    changed = True
                    n_fixed += 1
                new_list.append(inst)
            if changed:
                blk.instructions = new_list
    return n_fixed


def build_bass():
    nc = bass.Bass()

    q_ext = nc.declare_dram_parameter("q", [HEADS_PER_CORE, S, D], f32, isOutput=False)
    k_ext = nc.declare_dram_parameter("k", [HEADS_PER_CORE, S, D], f32, isOutput=False)
    v_ext = nc.declare_dram_parameter("v", [HEADS_PER_CORE, S, D], f32, isOutput=False)
    m_ext = nc.declare_dram_parameter("m", [HEADS_PER_CORE, S, S], u8, isOutput=False)
    outv = nc.declare_dram_parameter("out_v", [HEADS_PER_CORE, S, D], f32, isOutput=True)
    outp = nc.declare_dram_parameter("out_p", [HEADS_PER_CORE, S, S], f32, isOutput=True)

    with tile.TileContext(nc) as tc:
        with (
            tc.tile_pool(name="const", bufs=1) as const_pool,
            tc.tile_pool(name="head", bufs=2) as head_pool,
            tc.tile_pool(name="mask", bufs=3) as mask_pool,
            tc.tile_pool(name="sc", bufs=2) as sc_pool,
            tc.tile_pool(name="pn", bufs=2) as pn_pool,
            tc.tile_pool(name="at", bufs=2) as at_pool,
            tc.tile_pool(name="small", bufs=4) as small_pool,
            tc.tile_pool(name="vo", bufs=4) as vo_pool,
            tc.tile_pool(name="ps_qk", bufs=1, space="PSUM") as ps_qk,
            tc.tile_pool(name="ps_tr", bufs=2, space="PSUM") as ps_tr,
            tc.tile_pool(name="ps_av", bufs=2, space="PSUM") as ps_av,
        ):
            ident = const_pool.tile([P, P], f32)
            make_identity(nc, ident)
            ident_bf = const_pool.tile([P, P], bf16)
            nc.vector.tensor_copy(ident_bf, ident)

            for h in range(HEADS_PER_CORE):
                # ---- per-head prep: QT/KT [64, S] f32 (Q scaled), V bf16 ----
                q_nat = head_pool.tile([P, KC, D], f32, tag="q_nat")
                k_nat = head_pool.tile([P, KC, D], f32, tag="k_nat")
                v_nat = head_pool.tile([P, KC, D], f32, tag="v_nat")
                nc.sync.dma_start(
                    out=q_nat, in_=q_ext[h].rearrange("(t p) d -> p t d", p=P)
                )
                nc.sync.dma_start(
                    out=k_nat, in_=k_ext[h].rearrange("(t p) d -> p t d", p=P)
                )
                nc.sync.dma_start(
                    out=v_nat, in_=v_ext[h].rearrange("(t p) d -> p t d", p=P)
                )

                qt = head_pool.tile([D, S], bf16, tag="qt")
                kt = head_pool.tile([D, S], bf16, tag="kt")
                v_bf = head_pool.tile([P, KC, D], bf16, tag="v_bf")
                q_bf = head_pool.tile([P, KC, D], bf16, tag="q_bf")
                k_bf = head_pool.tile([P, KC, D], bf16, tag="k_bf")
                nc.vector.tensor_copy(v_bf, v_nat)
                # fold the 1/sqrt(D) scale into Q while casting to bf16
                nc.vector.tensor_scalar_mul(q_bf, q_nat, float(SCALE))
                nc.vector.tensor_copy(k_bf, k_nat)

                for t in range(0, KC, 4):
                    pt = ps_tr.tile([P, 1024], bf16, tag="ps_tr")
                    for j in range(4):
                        nc.tensor.transpose(
                            pt[:D, j * 256 : j * 256 + P], q_bf[:, t + j, :], ident_bf
                        )
                        nc.tensor.transpose(
                            pt[:D, j * 256 + P : j * 256 + 2 * P],
                            k_bf[:, t + j, :],
                            ident_bf,
                        )
                    nc.vector.tensor_copy(
                        qt[:, t * P : (t + 4) * P].rearrange("p (j c) -> p j c", j=4),
                        pt[:D].rearrange("p (j two c) -> p (j two) c", j=4, two=2)[
                            :, 0::2, :
                        ],
                    )
                    nc.vector.tensor_copy(
                        kt[:, t * P : (t + 4) * P].rearrange("p (j c) -> p j c", j=4),
                        pt[:D].rearrange("p (j two c) -> p (j two) c", j=4, two=2)[
                            :, 1::2, :
                        ],
                    )

                for g in range(NQT // GRP):
                    # corner-turn buffer: aT_g[p, kc, q_in_group]
                    at_g = at_pool.tile([P, KC, GRP * P], bf16, tag="at_g")

                    for gq in range(GRP):
                        qi = g * GRP + gq
                        qs = qi * P

                        m_tile = mask_pool.tile([P, S], u8, tag="m")
                        nc.sync.dma_start(out=m_tile, in_=m_ext[h, qs : qs + P, :])

                        qk = ps_qk.tile([P, S], f32, tag="qk")
                        for kj in range(4):
                            nc.tensor.matmul(
                                qk[:, kj * 512 : (kj + 1) * 512],
                                qt[:, qs : qs + P],
                                kt[:, kj * 512 : (kj + 1) * 512],
                                start=True,
                                stop=True,
                            )

                        # a = qk + mask * (-1e9)   (masked scores, f32)
                        a_t = sc_pool.tile([P, S], bf16, tag="a")
                        nc.vector.scalar_tensor_tensor(
                            out=a_t,
                            in0=m_tile,
                            scalar=NEG,
                            in1=qk,
                            op0=mybir.AluOpType.mult,
                            op1=mybir.AluOpType.add,
                        )

                        # p_unnorm = exp(a), rowsum via accum
                        p_un = pn_pool.tile([P, S], f32, tag="p_un")
                        rsum = small_pool.tile([P, 1], f32, tag="rsum")
                        nc.scalar.activation(
                            p_un,
                            a_t,
                            mybir.ActivationFunctionType.Exp,
                            accum_out=rsum,
                        )
                        rinv = small_pool.tile([P, 1], f32, tag="rinv")
                        nc.vector.reciprocal(rinv, rsum)

                        p_out = pn_pool.tile([P, S], f32, tag="p_out")
                        nc.gpsimd.tensor_scalar_mul(p_out, p_un, rinv)
                        nc.sync.dma_start(out=outp[h, qs : qs + P, :], in_=p_out)

                        # transpose a into the corner-turn buffer (bf16)
                        for kq in range(2):
                            pt = ps_tr.tile([P, 1024], bf16, tag="ps_tr")
                            for kk in range(8):
                                kj = kq * 8 + kk
                                nc.tensor.transpose(
                                    pt[:, kk * P : (kk + 1) * P],
                                    a_t[:, kj * P : (kj + 1) * P],
                                    ident_bf,
                                )
                            nc.scalar.copy(
                                at_g[:, kq * 8 : kq * 8 + 8, gq * P : (gq + 1) * P],
                                pt.rearrange("p (a b) -> p a b", a=8),
                            )

                    # ---- AV for this group: attn_vT[64, GRP*P] ----
                    av = ps_av.tile([D, GRP * P], f32, tag="av")
                    for kj in range(KC):
                        nc.tensor.matmul(
                            av,
                            v_bf[:, kj, :],
                            at_g[:, kj, :],
                            start=(kj == 0),
                            stop=(kj == KC - 1),
                        )
                    av_sb = vo_pool.tile([D, GRP * P], bf16, tag="av_sb")
                    nc.vector.tensor_copy(av_sb, av)
                    # transpose back to [q, D] and write out
                    pt2 = ps_tr.tile([P, 1024], bf16, tag="ps_tr")
                    for gq in range(GRP):
                        nc.tensor.transpose(
                            pt2[:, gq * 256 : gq * 256 + D],
                            av_sb[:, gq * P : (gq + 1) * P],
                            ident_bf[:D, :D],
                        )
                    vo_sb = vo_pool.tile([P, GRP, D], f32, tag="vo_sb")
                    nc.vector.tensor_copy(
                        vo_sb,
                        pt2.rearrange("p (g c) -> p g c", g=4)[:, :, :D],
                    )
                    qs = g * GRP * P
                    nc.sync.dma_start(
                        out=outv[h, qs : qs + GRP * P, :].rearrange(
                            "(g p) d -> p g d", p=P
                        ),
                        in_=vo_sb,
                    )

    legalize_waits(nc)
    return nc


def kernel(Q, K, V, attn_mask):
    Q = np.ascontiguousarray(np.asarray(Q), dtype=np.float32)
    K = np.ascontiguousarray(np.asarray(K), dtype=np.float32)
    V = np.ascontiguousarray(np.asarray(V), dtype=np.float32)
    M = np.asarray(attn_mask).astype(np.uint8)

    qf = Q.reshape(B * H, S, D)
    kf = K.reshape(B * H, S, D)
    vf = V.reshape(B * H, S, D)
    mf = M.reshape(B * H, S, S)

    if "nc" not in _COMPILED:
        _COMPILED["nc"] = build_bass()
    nc = _COMPILED["nc"]

    in_maps = []
    for c in range(N_CORES):
        sl = slice(c * HEADS_PER_CORE, (c + 1) * HEADS_PER_CORE)
        in_maps.append(
            {
                "q": np.ascontiguousarray(qf[sl]),
                "k": np.ascontiguousarray(kf[sl]),
                "v": np.ascontiguousarray(vf[sl]),
                "m": np.ascontiguousarray(mf[sl]),
            }
        )

    res = run_bass_kernel_spmd(nc, in_maps, core_ids=list(range(N_CORES)))
    results = res.results

    attn_v = np.concatenate([results[c]["out_v"] for c in range(N_CORES)], axis=0)
    attn_p = np.concatenate([results[c]["out_p"] for c in range(N_CORES)], axis=0)
    attn_v = attn_v.reshape(B, H, S, D).astype(np.float32)
    attn_p = attn_p.reshape(B, H, S, S).astype(np.float32)
    return attn_v, attn_p


if __name__ == "__main__":
    rng = np.random.default_rng(0)
    Q = rng.standard_normal((B, H, S, D), dtype=np.float32)
    K = rng.standard_normal((B, H, S, D), dtype=np.float32)
    V = rng.standard_normal((B, H, S, D), dtype=np.float32)
    Mm = rng.integers(0, 2, size=(B, H, S, S)).astype(bool)
    av, ap = kernel(Q, K, V, Mm)
    print(av.shape, ap.shape, av.dtype, ap.dtype)


# revision 9
# speedup vs baseline: 4.3181x; 4.3181x over previous
"""Trainium2 Bass kernel for masked attention with pre-softmax-score AV matmul.

Reference semantics (faithful to the source module's bug):
    a = (Q @ K^T) / sqrt(D);  a = where(mask, -1e9, a)
    attn_p = softmax(a, axis=-1)
    attn_v = a @ V            # uses pre-softmax masked scores, NOT attn_p
    returns (attn_v, attn_p)

Shapes: Q,K,V (2,8,2048,64) f32; mask (2,8,2048,2048) bool.
Sharding: B*H = 16 head-slices, 2 per core across 8 cores (pure data
parallel, no collectives).
"""

import sys

sys.path.insert(0, "/opt/trn_rl_repo")

import numpy as np

import concourse.bass as bass
import concourse.tile as tile
from concourse import mybir
from concourse.bass_utils import run_bass_kernel_spmd
from concourse.masks import make_identity

B, H, S, D = 2, 8, 2048, 64
N_CORES = 8
HEADS_PER_CORE = (B * H) // N_CORES  # 2
P = 128                              # partition tile (q rows per tile)
NQT = S // P                         # 16 q-tiles per head
KC = S // P                          # 16 k-chunks of 128
GRP = 4                              # q-tiles per AV matmul group
SCALE = 1.0 / np.sqrt(np.float32(D))  # 0.125, exact in fp32
NEG = -1.0e9

f32 = mybir.dt.float32
f32r = mybir.dt.float32r
bf16 = mybir.dt.bfloat16
u8 = mybir.dt.uint8

_COMPILED = {}


def legalize_waits(nc):
    """This walrus build allows only ONE sync-wait command per instruction.

    Tile's wait assigner can attach several (one per upstream engine), which
    fails codegen with 'Too many sync wait commands'. Hoist all but the last
    wait onto preceding same-engine NoOps (program order on the engine's
    sequencer makes this semantically identical).
    """
    n_fixed = 0
    for fn in nc.m.functions:
        for blk in fn.blocks:
            insts = list(blk.instructions)
            new_list = []
            changed = False
            for inst in insts:
                si = inst.sync_info
                if si is not None and si.on_wait is not None and len(si.on_wait) > 1:
                    waits = list(si.on_wait)
                    for wi, w in enumerate(waits[:-1]):
                        new_list.append(
                            mybir.InstNoOp(
                                name=f"{inst.name}-wn{wi}",
                                engine=inst.engine,
                                sync_info=mybir.SyncInfo(on_wait=[w], on_update=[]),
                            )
                        )
                    inst.sync_info = mybir.SyncInfo(
                        on_wait=waits[-1:], on_update=list(si.on_update or [])
                    )
                    changed = True
                    n_fixed += 1
                new_list.append(inst)
            if changed:
                blk.instructions = new_list
    return n_fixed


def build_bass():
    nc = bass.Bass()

    q_ext = nc.declare_dram_parameter("q", [HEADS_PER_CORE, S, D], f32, isOutput=False)
    k_ext = nc.declare_dram_parameter("k", [HEADS_PER_CORE, S, D], f32, isOutput=False)
    v_ext = nc.declare_dram_parameter("v", [HEADS_PER_CORE, S, D], f32, isOutput=False)
    m_ext = nc.declare_dram_parameter("m", [HEADS_PER_CORE, S, S], u8, isOutput=False)
    outv = nc.declare_dram_parameter("out_v", [HEADS_PER_CORE, S, D], f32, isOutput=True)
    outp = nc.declare_dram_parameter("out_p", [HEADS_PER_CORE, S, S], f32, isOutput=True)

    with tile.TileContext(nc) as tc:
        with (
            tc.tile_pool(name="const", bufs=1) as const_pool,
            tc.tile_pool(name="head", bufs=2) as head_pool,
            tc.tile_pool(name="mask", bufs=3) as mask_pool,
            tc.tile_pool(name="sc", bufs=2) as sc_pool,
            tc.tile_pool(name="pn", bufs=2) as pn_pool,
            tc.tile_pool(name="at", bufs=2) as at_pool,
            tc.tile_pool(name="small", bufs=4) as small_pool,
            tc.tile_pool(name="vo", bufs=4) as vo_pool,
            tc.tile_pool(name="ps_qk", bufs=1, space="PSUM") as ps_qk,
            tc.tile_pool(name="ps_tr", bufs=2, space="PSUM") as ps_tr,
            tc.tile_pool(name="ps_av", bufs=2, space="PSUM") as ps_av,
        ):
            ident = const_pool.tile([P, P], f32)
            make_identity(nc, ident)
            ident_bf = const_pool.tile([P, P], bf16)
            nc.vector.tensor_copy(ident_bf, ident)

            for h in range(HEADS_PER_CORE):
                # ---- per-head prep: QT/KT [64, S] f32 (Q scaled), V bf16 ----
                q_nat = head_pool.tile([P, KC, D], f32, tag="q_nat")
                k_nat = head_pool.tile([P, KC, D], f32, tag="k_nat")
                v_nat = head_pool.tile([P, KC, D], f32, tag="v_nat")
                nc.sync.dma_start(
                    out=q_nat, in_=q_ext[h].rearrange("(t p) d -> p t d", p=P)
                )
                nc.sync.dma_start(
                    out=k_nat, in_=k_ext[h].rearrange("(t p) d -> p t d", p=P)
                )
                nc.sync.dma_start(
                    out=v_nat, in_=v_ext[h].rearrange("(t p) d -> p t d", p=P)
                )

                qt = head_pool.tile([D, S], bf16, tag="qt")
                kt = head_pool.tile([D, S], bf16, tag="kt")
                v_bf = head_pool.tile([P, KC, D], bf16, tag="v_bf")
                q_bf = head_pool.tile([P, KC, D], bf16, tag="q_bf")
                k_bf = head_pool.tile([P, KC, D], bf16, tag="k_bf")
                nc.vector.tensor_copy(v_bf, v_nat)
                # fold the 1/sqrt(D) scale into Q while casting to bf16
                nc.vector.tensor_scalar_mul(q_bf, q_nat, float(SCALE))
                nc.vector.tensor_copy(k_bf, k_nat)

                for t in range(0, KC, 4):
                    pt = ps_tr.tile([P, 1024], bf16, tag="ps_tr")
                    for j in range(4):
                        nc.tensor.transpose(
                            pt[:D, j * P : (j + 1) * P], q_bf[:, t + j, :], ident_bf
                        )
                        nc.tensor.transpose(
                            pt[:D, 512 + j * P : 512 + (j + 1) * P],
                            k_bf[:, t + j, :],
                            ident_bf,
                        )
                    nc.vector.tensor_copy(
                        qt[:, t * P : (t + 4) * P], pt[:D, 0:512]
                    )
                    nc.vector.tensor_copy(
                        kt[:, t * P : (t + 4) * P], pt[:D, 512:1024]
                    )

                for g in range(NQT // GRP):
                    # corner-turn buffer: aT_g[p, kc, q_in_group]
                    at_g = at_pool.tile([P, KC, GRP * P], bf16, tag="at_g")

                    for gq in range(GRP):
                        qi = g * GRP + gq
                        qs = qi * P

                        m_tile = mask_pool.tile([P, S], u8, tag="m")
                        nc.sync.dma_start(out=m_tile, in_=m_ext[h, qs : qs + P, :])

                        qk = ps_qk.tile([P, S], f32, tag="qk")
                        for kj in range(4):
                            nc.tensor.matmul(
                                qk[:, kj * 512 : (kj + 1) * 512],
                                qt[:, qs : qs + P],
                                kt[:, kj * 512 : (kj + 1) * 512],
                                start=True,
                                stop=True,
                            )

                        # a = qk + mask * (-1e9)   (masked scores, f32)
                        a_t = sc_pool.tile([P, S], bf16, tag="a")
                        nc.vector.scalar_tensor_tensor(
                            out=a_t,
                            in0=m_tile,
                            scalar=NEG,
                            in1=qk,
                            op0=mybir.AluOpType.mult,
                            op1=mybir.AluOpType.add,
                        )

                        # p_unnorm = exp(a), rowsum via accum
                        p_un = pn_pool.tile([P, S], f32, tag="p_un")
                        rsum = small_pool.tile([P, 1], f32, tag="rsum")
                        nc.scalar.activation(
                            p_un,
                            a_t,
                            mybir.ActivationFunctionType.Exp,
                            accum_out=rsum,
                        )
                        rinv = small_pool.tile([P, 1], f32, tag="rinv")
                        nc.vector.reciprocal(rinv, rsum)

                        p_out = pn_pool.tile([P, S], f32, tag="p_out")
                        nc.vector.tensor_scalar_mul(p_out, p_un, rinv)
                        nc.sync.dma_start(out=outp[h, qs : qs + P, :], in_=p_out)

                        # transpose a into the corner-turn buffer (bf16)
                        for kq in range(2):
                            pt = ps_tr.tile([P, 1024], bf16, tag="ps_tr")
                            for kk in range(8):
                                kj = kq * 8 + kk
                                nc.tensor.transpose(
                                    pt[:, kk * P : (kk + 1) * P],
                                    a_t[:, kj * P : (kj + 1) * P],
                                    ident_bf,
                                )
                            nc.scalar.copy(
                                at_g[:, kq * 8 : kq * 8 + 8, gq * P : (gq + 1) * P],
                                pt.rearrange("p (a b) -> p a b", a=8),
                            )

                    # ---- AV for this group: attn_vT[64, GRP*P] ----
                    av = ps_av.tile([D, GRP * P], f32, tag="av")
                    for kj in range(KC):
                        nc.tensor.matmul(
                            av,
                            v_bf[:, kj, :],
                            at_g[:, kj, :],
                            start=(kj == 0),
                            stop=(kj == KC - 1),
                        )
                    av_sb = vo_pool.tile([D, GRP * P], bf16, tag="av_sb")
                    nc.vector.tensor_copy(av_sb, av)
                    # transpose back to [q, D] and write out
                    pt2 = ps_tr.tile([P, 1024], bf16, tag="ps_tr")
                    for gq in range(GRP):
                        nc.tensor.transpose(
                            pt2[:, gq * 256 : gq * 256 + D],
                            av_sb[:, gq * P : (gq + 1) * P],
                            ident_bf[:D, :D],
                        )
                    vo_sb = vo_pool.tile([P, GRP, D], f32, tag="vo_sb")
                    nc.vector.tensor_copy(
                        vo_sb,
                        pt2.rearrange("p (g c) -> p g c", g=4)[:, :, :D],
                    )
                    qs = g * GRP * P
                    nc.sync.dma_start(
                        out=outv[h, qs : qs + GRP * P, :].rearrange(
                            "(g p) d -> p g d", p=P
                        ),
                        in_=vo_sb,
                    )

    legalize_waits(nc)
    return nc


def kernel(Q, K, V, attn_mask):
    Q = np.ascontiguousarray(np.asarray(Q), dtype=np.float32)
    K = np.ascontiguousarray(np.asarray(K), dtype=np.float32)
    V = np.ascontiguousarray(np.asarray(V), dtype=np.float32)
    M = np.asarray(attn_mask).astype(np.uint8)

    qf = Q.reshape(B * H, S, D)
    kf = K.reshape(B * H, S, D)
    vf = V.reshape(B * H, S, D)
    mf = M.reshape(B * H, S, S)

    if "nc" not in _COMPILED:
        _COMPILED["nc"] = build_bass()
    nc = _COMPILED["nc"]

    in_maps = []
    for c in range(N_CORES):
        sl = slice(c * HEADS_PER_CORE, (c + 1) * HEADS_PER_CORE)
        in_maps.append(
            {
                "q": np.ascontiguousarray(qf[sl]),
                "k": np.ascontiguousarray(kf[sl]),
                "v": np.ascontiguousarray(vf[sl]),
                "m": np.ascontiguousarray(mf[sl]),
            }
        )

    res = run_bass_kernel_spmd(nc, in_maps, core_ids=list(range(N_CORES)))
    results = res.results

    attn_v = np.concatenate([results[c]["out_v"] for c in range(N_CORES)], axis=0)
    attn_p = np.concatenate([results[c]["out_p"] for c in range(N_CORES)], axis=0)
    attn_v = attn_v.reshape(B, H, S, D).astype(np.float32)
    attn_p = attn_p.reshape(B, H, S, S).astype(np.float32)
    return attn_v, attn_p


if __name__ == "__main__":
    rng = np.random.default_rng(0)
    Q = rng.standard_normal((B, H, S, D), dtype=np.float32)
    K = rng.standard_normal((B, H, S, D), dtype=np.float32)
    V = rng.standard_normal((B, H, S, D), dtype=np.float32)
    Mm = rng.integers(0, 2, size=(B, H, S, S)).astype(bool)
    av, ap = kernel(Q, K, V, Mm)
    print(av.shape, ap.shape, av.dtype, ap.dtype)


# revision 10
# speedup vs baseline: 4.7251x; 1.0942x over previous
"""Trainium2 Bass kernel for masked attention with pre-softmax-score AV matmul.

Reference semantics (faithful to the source module's bug):
    a = (Q @ K^T) / sqrt(D);  a = where(mask, -1e9, a)
    attn_p = softmax(a, axis=-1)
    attn_v = a @ V            # uses pre-softmax masked scores, NOT attn_p
    returns (attn_v, attn_p)

Shapes: Q,K,V (2,8,2048,64) f32; mask (2,8,2048,2048) bool.
Sharding: B*H = 16 head-slices, 2 per core across 8 cores (pure data
parallel, no collectives).
"""

import sys

sys.path.insert(0, "/opt/trn_rl_repo")

import numpy as np

import concourse.bass as bass
import concourse.tile as tile
from concourse import mybir
from concourse.bass_utils import run_bass_kernel_spmd
from concourse.masks import make_identity

B, H, S, D = 2, 8, 2048, 64
N_CORES = 8
HEADS_PER_CORE = (B * H) // N_CORES  # 2
P = 128                              # partition tile (q rows per tile)
NQT = S // P                         # 16 q-tiles per head
KC = S // P                          # 16 k-chunks of 128
GRP = 4                              # q-tiles per AV matmul group
SCALE = 1.0 / np.sqrt(np.float32(D))  # 0.125, exact in fp32
NEG = -1.0e9

f32 = mybir.dt.float32
f32r = mybir.dt.float32r
bf16 = mybir.dt.bfloat16
u8 = mybir.dt.uint8

_COMPILED = {}


def legalize_waits(nc):
    """This walrus build allows only ONE sync-wait command per instruction.

    Tile's wait assigner can attach several (one per upstream engine), which
    fails codegen with 'Too many sync wait commands'. Hoist all but the last
    wait onto preceding same-engine NoOps (program order on the engine's
    sequencer makes this semantically identical).
    """
    n_fixed = 0
    for fn in nc.m.functions:
        for blk in fn.blocks:
            insts = list(blk.instructions)
            new_list = []
            changed = False
            for inst in insts:
                si = inst.sync_info
                if si is not None and si.on_wait is not None and len(si.on_wait) > 1:
                    waits = list(si.on_wait)
                    for wi, w in enumerate(waits[:-1]):
                        new_list.append(
                            mybir.InstNoOp(
                                name=f"{inst.name}-wn{wi}",
                                engine=inst.engine,
                                sync_info=mybir.SyncInfo(on_wait=[w], on_update=[]),
                            )
                        )
                    inst.sync_info = mybir.SyncInfo(
                        on_wait=waits[-1:], on_update=list(si.on_update or [])
                    )
                    changed = True
                    n_fixed += 1
                new_list.append(inst)
            if changed:
                blk.instructions = new_list
    return n_fixed


def build_bass():
    nc = bass.Bass()

    q_ext = nc.declare_dram_parameter("q", [HEADS_PER_CORE, S, D], f32, isOutput=False)
    k_ext = nc.declare_dram_parameter("k", [HEADS_PER_CORE, S, D], f32, isOutput=False)
    v_ext = nc.declare_dram_parameter("v", [HEADS_PER_CORE, S, D], f32, isOutput=False)
    m_ext = nc.declare_dram_parameter("m", [HEADS_PER_CORE, S, S], u8, isOutput=False)
    outv = nc.declare_dram_parameter("out_v", [HEADS_PER_CORE, S, D], f32, isOutput=True)
    outp = nc.declare_dram_parameter("out_p", [HEADS_PER_CORE, S, S], f32, isOutput=True)

    with tile.TileContext(nc) as tc:
        with (
            tc.tile_pool(name="const", bufs=1) as const_pool,
            tc.tile_pool(name="head", bufs=2) as head_pool,
            tc.tile_pool(name="mask", bufs=4) as mask_pool,
            tc.tile_pool(name="sc", bufs=3) as sc_pool,
            tc.tile_pool(name="pn", bufs=3) as pn_pool,
            tc.tile_pool(name="at", bufs=2) as at_pool,
            tc.tile_pool(name="small", bufs=4) as small_pool,
            tc.tile_pool(name="vo", bufs=4) as vo_pool,
            tc.tile_pool(name="ps_qk", bufs=2, space="PSUM") as ps_qk,
            tc.tile_pool(name="ps_tr", bufs=2, space="PSUM") as ps_tr,
            tc.tile_pool(name="ps_av", bufs=2, space="PSUM") as ps_av,
        ):
            ident = const_pool.tile([P, P], f32)
            make_identity(nc, ident)
            ident_bf = const_pool.tile([P, P], bf16)
            nc.vector.tensor_copy(ident_bf, ident)

            for h in range(HEADS_PER_CORE):
                # ---- per-head prep: QT/KT [64, S] f32 (Q scaled), V bf16 ----
                q_nat = head_pool.tile([P, KC, D], f32, tag="q_nat")
                k_nat = head_pool.tile([P, KC, D], f32, tag="k_nat")
                v_nat = head_pool.tile([P, KC, D], f32, tag="v_nat")
                nc.sync.dma_start(
                    out=q_nat, in_=q_ext[h].rearrange("(t p) d -> p t d", p=P)
                )
                nc.sync.dma_start(
                    out=k_nat, in_=k_ext[h].rearrange("(t p) d -> p t d", p=P)
                )
                nc.sync.dma_start(
                    out=v_nat, in_=v_ext[h].rearrange("(t p) d -> p t d", p=P)
                )

                qt = head_pool.tile([D, S], bf16, tag="qt")
                kt = head_pool.tile([D, S], bf16, tag="kt")
                v_bf = head_pool.tile([P, KC, D], bf16, tag="v_bf")
                q_bf = head_pool.tile([P, KC, D], bf16, tag="q_bf")
                k_bf = head_pool.tile([P, KC, D], bf16, tag="k_bf")
                nc.vector.tensor_copy(v_bf, v_nat)
                # fold the 1/sqrt(D) scale into Q while casting to bf16
                nc.vector.tensor_scalar_mul(q_bf, q_nat, float(SCALE))
                nc.vector.tensor_copy(k_bf, k_nat)

                for t in range(0, KC, 4):
                    pt = ps_tr.tile([P, 1024], bf16, tag="ps_tr")
                    for j in range(4):
                        nc.tensor.transpose(
                            pt[:D, j * P : (j + 1) * P], q_bf[:, t + j, :], ident_bf
                        )
                        nc.tensor.transpose(
                            pt[:D, 512 + j * P : 512 + (j + 1) * P],
                            k_bf[:, t + j, :],
                            ident_bf,
                        )
                    nc.vector.tensor_copy(
                        qt[:, t * P : (t + 4) * P], pt[:D, 0:512]
                    )
                    nc.vector.tensor_copy(
                        kt[:, t * P : (t + 4) * P], pt[:D, 512:1024]
                    )

                for g in range(NQT // GRP):
                    # corner-turn buffer: aT_g[p, kc, q_in_group]
                    at_g = at_pool.tile([P, KC, GRP * P], bf16, tag="at_g")

                    for gq in range(GRP):
                        qi = g * GRP + gq
                        qs = qi * P

                        m_tile = mask_pool.tile([P, S], u8, tag="m")
                        nc.sync.dma_start(out=m_tile, in_=m_ext[h, qs : qs + P, :])

                        # a = qk + mask * (-1e9)  (masked scores, bf16),
                        # in two pipelined halves so PE can start the next
                        # half/tile while DVE drains the previous one
                        a_t = sc_pool.tile([P, S], bf16, tag="a")
                        for half in range(2):
                            hs = half * 1024
                            qk = ps_qk.tile([P, 1024], f32, tag="qk")
                            for kj in range(2):
                                nc.tensor.matmul(
                                    qk[:, kj * 512 : (kj + 1) * 512],
                                    qt[:, qs : qs + P],
                                    kt[:, hs + kj * 512 : hs + (kj + 1) * 512],
                                    start=True,
                                    stop=True,
                                )
                            nc.vector.scalar_tensor_tensor(
                                out=a_t[:, hs : hs + 1024],
                                in0=m_tile[:, hs : hs + 1024],
                                scalar=NEG,
                                in1=qk,
                                op0=mybir.AluOpType.mult,
                                op1=mybir.AluOpType.add,
                            )

                        # p_unnorm = exp(a), rowsum via accum
                        p_un = pn_pool.tile([P, S], f32, tag="p_un")
                        rsum = small_pool.tile([P, 1], f32, tag="rsum")
                        nc.scalar.activation(
                            p_un,
                            a_t,
                            mybir.ActivationFunctionType.Exp,
                            accum_out=rsum,
                        )
                        rinv = small_pool.tile([P, 1], f32, tag="rinv")
                        nc.vector.reciprocal(rinv, rsum)

                        p_out = pn_pool.tile([P, S], f32, tag="p_out")
                        nc.vector.tensor_scalar_mul(p_out, p_un, rinv)
                        nc.sync.dma_start(out=outp[h, qs : qs + P, :], in_=p_out)

                        # transpose a into the corner-turn buffer (bf16)
                        for kq in range(2):
                            pt = ps_tr.tile([P, 1024], bf16, tag="ps_tr")
                            for kk in range(8):
                                kj = kq * 8 + kk
                                nc.tensor.transpose(
                                    pt[:, kk * P : (kk + 1) * P],
                                    a_t[:, kj * P : (kj + 1) * P],
                                    ident_bf,
                                )
                            nc.scalar.copy(
                                at_g[:, kq * 8 : kq * 8 + 8, gq * P : (gq + 1) * P],
                                pt.rearrange("p (a b) -> p a b", a=8),
                            )

                    # ---- AV for this group: attn_vT[64, GRP*P] ----
                    av = ps_av.tile([D, GRP * P], f32, tag="av")
                    for kj in range(KC):
                        nc.tensor.matmul(
                            av,
                            v_bf[:, kj, :],
                            at_g[:, kj, :],
                            start=(kj == 0),
                            stop=(kj == KC - 1),
                        )
                    av_sb = vo_pool.tile([D, GRP * P], bf16, tag="av_sb")
                    nc.vector.tensor_copy(av_sb, av)
                    # transpose back to [q, D] and write out
                    pt2 = ps_tr.tile([P, 1024], bf16, tag="ps_tr")
                    for gq in range(GRP):
                        nc.tensor.transpose(
                            pt2[:, gq * 256 : gq * 256 + D],
                            av_sb[:, gq * P : (gq + 1) * P],
                            ident_bf[:D, :D],
                        )
                    vo_sb = vo_pool.tile([P, GRP, D], f32, tag="vo_sb")
                    nc.vector.tensor_copy(
                        vo_sb,
                        pt2.rearrange("p (g c) -> p g c", g=4)[:, :, :D],
                    )
                    qs = g * GRP * P
                    nc.sync.dma_start(
                        out=outv[h, qs : qs + GRP * P, :].rearrange(
                            "(g p) d -> p g d", p=P
                        ),
                        in_=vo_sb,
                    )

    legalize_waits(nc)
    return nc


def kernel(Q, K, V, attn_mask):
    Q = np.ascontiguousarray(np.asarray(Q), dtype=np.float32)
    K = np.ascontiguousarray(np.asarray(K), dtype=np.float32)
    V = np.ascontiguousarray(np.asarray(V), dtype=np.float32)
    M = np.asarray(attn_mask).astype(np.uint8)

    qf = Q.reshape(B * H, S, D)
    kf = K.reshape(B * H, S, D)
    vf = V.reshape(B * H, S, D)
    mf = M.reshape(B * H, S, S)

    if "nc" not in _COMPILED:
        _COMPILED["nc"] = build_bass()
    nc = _COMPILED["nc"]

    in_maps = []
    for c in range(N_CORES):
        sl = slice(c * HEADS_PER_CORE, (c + 1) * HEADS_PER_CORE)
        in_maps.append(
            {
                "q": np.ascontiguousarray(qf[sl]),
                "k": np.ascontiguousarray(kf[sl]),
                "v": np.ascontiguousarray(vf[sl]),
                "m": np.ascontiguousarray(mf[sl]),
            }
        )

    res = run_bass_kernel_spmd(nc, in_maps, core_ids=list(range(N_CORES)))
    results = res.results

    attn_v = np.concatenate([results[c]["out_v"] for c in range(N_CORES)], axis=0)
    attn_p = np.concatenate([results[c]["out_p"] for c in range(N_CORES)], axis=0)
    attn_v = attn_v.reshape(B, H, S, D).astype(np.float32)
    attn_p = attn_p.reshape(B, H, S, S).astype(np.float32)
    return attn_v, attn_p


if __name__ == "__main__":
    rng = np.random.default_rng(0)
    Q = rng.standard_normal((B, H, S, D), dtype=np.float32)
    K = rng.standard_normal((B, H, S, D), dtype=np.float32)
    V = rng.standard_normal((B, H, S, D), dtype=np.float32)
    Mm = rng.integers(0, 2, size=(B, H, S, S)).astype(bool)
    av, ap = kernel(Q, K, V, Mm)
    print(av.shape, ap.shape, av.dtype, ap.dtype)


# revision 11
# speedup vs baseline: 5.0696x; 1.0729x over previous
"""Trainium2 Bass kernel for masked attention with pre-softmax-score AV matmul.

Reference semantics (faithful to the source module's bug):
    a = (Q @ K^T) / sqrt(D);  a = where(mask, -1e9, a)
    attn_p = softmax(a, axis=-1)
    attn_v = a @ V            # uses pre-softmax masked scores, NOT attn_p
    returns (attn_v, attn_p)

Shapes: Q,K,V (2,8,2048,64) f32; mask (2,8,2048,2048) bool.
Sharding: B*H = 16 head-slices, 2 per core across 8 cores (pure data
parallel, no collectives).
"""

import sys

sys.path.insert(0, "/opt/trn_rl_repo")

import numpy as np

import concourse.bass as bass
import concourse.tile as tile
from concourse import mybir
from concourse.bass_utils import run_bass_kernel_spmd
from concourse.masks import make_identity

B, H, S, D = 2, 8, 2048, 64
N_CORES = 8
HEADS_PER_CORE = (B * H) // N_CORES  # 2
P = 128                              # partition tile (q rows per tile)
NQT = S // P                         # 16 q-tiles per head
KC = S // P                          # 16 k-chunks of 128
GRP = 4                              # q-tiles per AV matmul group
SCALE = 1.0 / np.sqrt(np.float32(D))  # 0.125, exact in fp32
NEG = -1.0e9

f32 = mybir.dt.float32
f32r = mybir.dt.float32r
bf16 = mybir.dt.bfloat16
u8 = mybir.dt.uint8

_COMPILED = {}


def legalize_waits(nc):
    """This walrus build allows only ONE sync-wait command per instruction.

    Tile's wait assigner can attach several (one per upstream engine), which
    fails codegen with 'Too many sync wait commands'. Hoist all but the last
    wait onto preceding same-engine NoOps (program order on the engine's
    sequencer makes this semantically identical).
    """
    n_fixed = 0
    for fn in nc.m.functions:
        for blk in fn.blocks:
            insts = list(blk.instructions)
            new_list = []
            changed = False
            for inst in insts:
                si = inst.sync_info
                if si is not None and si.on_wait is not None and len(si.on_wait) > 1:
                    waits = list(si.on_wait)
                    for wi, w in enumerate(waits[:-1]):
                        new_list.append(
                            mybir.InstNoOp(
                                name=f"{inst.name}-wn{wi}",
                                engine=inst.engine,
                                sync_info=mybir.SyncInfo(on_wait=[w], on_update=[]),
                            )
                        )
                    inst.sync_info = mybir.SyncInfo(
                        on_wait=waits[-1:], on_update=list(si.on_update or [])
                    )
                    changed = True
                    n_fixed += 1
                new_list.append(inst)
            if changed:
                blk.instructions = new_list
    return n_fixed


def build_bass():
    nc = bass.Bass()

    q_ext = nc.declare_dram_parameter("q", [HEADS_PER_CORE, S, D], f32, isOutput=False)
    k_ext = nc.declare_dram_parameter("k", [HEADS_PER_CORE, S, D], f32, isOutput=False)
    v_ext = nc.declare_dram_parameter("v", [HEADS_PER_CORE, S, D], f32, isOutput=False)
    m_ext = nc.declare_dram_parameter("m", [HEADS_PER_CORE, S, S], u8, isOutput=False)
    outv = nc.declare_dram_parameter("out_v", [HEADS_PER_CORE, S, D], f32, isOutput=True)
    outp = nc.declare_dram_parameter("out_p", [HEADS_PER_CORE, S, S], f32, isOutput=True)

    with tile.TileContext(nc) as tc:
        with (
            tc.tile_pool(name="const", bufs=1) as const_pool,
            tc.tile_pool(name="head", bufs=2) as head_pool,
            tc.tile_pool(name="mask", bufs=4) as mask_pool,
            tc.tile_pool(name="sc", bufs=3) as sc_pool,
            tc.tile_pool(name="pn", bufs=3) as pn_pool,
            tc.tile_pool(name="at", bufs=2) as at_pool,
            tc.tile_pool(name="small", bufs=4) as small_pool,
            tc.tile_pool(name="vo", bufs=4) as vo_pool,
            tc.tile_pool(name="ps_qk", bufs=2, space="PSUM") as ps_qk,
            tc.tile_pool(name="ps_tr", bufs=2, space="PSUM") as ps_tr,
            tc.tile_pool(name="ps_av", bufs=2, space="PSUM") as ps_av,
        ):
            ident = const_pool.tile([P, P], f32)
            make_identity(nc, ident)
            ident_bf = const_pool.tile([P, P], bf16)
            nc.vector.tensor_copy(ident_bf, ident)

            for h in range(HEADS_PER_CORE):
                # ---- per-head prep: QT/KT [64, S] f32 (Q scaled), V bf16 ----
                q_nat = head_pool.tile([P, KC, D], f32, tag="q_nat")
                k_nat = head_pool.tile([P, KC, D], f32, tag="k_nat")
                v_nat = head_pool.tile([P, KC, D], f32, tag="v_nat")
                nc.sync.dma_start(
                    out=q_nat, in_=q_ext[h].rearrange("(t p) d -> p t d", p=P)
                )
                nc.sync.dma_start(
                    out=k_nat, in_=k_ext[h].rearrange("(t p) d -> p t d", p=P)
                )
                nc.sync.dma_start(
                    out=v_nat, in_=v_ext[h].rearrange("(t p) d -> p t d", p=P)
                )

                qt = head_pool.tile([2 * D, S], bf16, tag="qt")
                kt = head_pool.tile([2 * D, S], bf16, tag="kt")
                v_bf = head_pool.tile([P, KC, D], bf16, tag="v_bf")
                q_bf = head_pool.tile([P, KC, D], bf16, tag="q_bf")
                k_bf = head_pool.tile([P, KC, D], bf16, tag="k_bf")
                nc.vector.tensor_copy(v_bf, v_nat)
                # fold the 1/sqrt(D) scale into Q while casting to bf16
                nc.vector.tensor_scalar_mul(q_bf, q_nat, float(SCALE))
                nc.vector.tensor_copy(k_bf, k_nat)

                for t in range(0, KC, 4):
                    pt = ps_tr.tile([P, 1024], bf16, tag="ps_tr")
                    for j in range(4):
                        nc.tensor.transpose(
                            pt[:D, j * P : (j + 1) * P], q_bf[:, t + j, :], ident_bf
                        )
                        nc.tensor.transpose(
                            pt[:D, 512 + j * P : 512 + (j + 1) * P],
                            k_bf[:, t + j, :],
                            ident_bf,
                        )
                    nc.vector.tensor_copy(
                        qt[:D, t * P : (t + 4) * P], pt[:D, 0:512]
                    )
                    nc.vector.tensor_copy(
                        qt[D:, t * P : (t + 4) * P], pt[:D, 0:512]
                    )
                    nc.vector.tensor_copy(
                        kt[:D, t * P : (t + 4) * P], pt[:D, 512:1024]
                    )
                    nc.vector.tensor_copy(
                        kt[D:, t * P : (t + 4) * P], pt[:D, 512:1024]
                    )

                for g in range(NQT // GRP):
                    # corner-turn buffer: aT_g[p, kc, q_in_group]
                    at_g = at_pool.tile([P, KC, GRP * P], bf16, tag="at_g")

                    for gq in range(GRP):
                        qi = g * GRP + gq
                        qs = qi * P

                        m_tile = mask_pool.tile([P, S], u8, tag="m")
                        nc.sync.dma_start(out=m_tile, in_=m_ext[h, qs : qs + P, :])

                        # a = qk + mask * (-1e9)  (masked scores, bf16),
                        # in two pipelined halves so PE can start the next
                        # half/tile while DVE drains the previous one
                        a_t = sc_pool.tile([P, S], bf16, tag="a")
                        for half in range(2):
                            hs = half * 1024
                            qk = ps_qk.tile([P, 1024], f32, tag="qk")
                            nc.tensor.matmul(
                                qk[:, 0:512],
                                qt[:D, qs : qs + P],
                                kt[:D, hs : hs + 512],
                                start=True,
                                stop=True,
                            )
                            nc.tensor.matmul(
                                qk[:, 512:1024],
                                qt[D:, qs : qs + P],
                                kt[D:, hs + 512 : hs + 1024],
                                start=True,
                                stop=True,
                            )
                            nc.vector.scalar_tensor_tensor(
                                out=a_t[:, hs : hs + 1024],
                                in0=m_tile[:, hs : hs + 1024],
                                scalar=NEG,
                                in1=qk,
                                op0=mybir.AluOpType.mult,
                                op1=mybir.AluOpType.add,
                            )

                        # p_unnorm = exp(a), rowsum via accum
                        p_un = pn_pool.tile([P, S], f32, tag="p_un")
                        rsum = small_pool.tile([P, 1], f32, tag="rsum")
                        nc.scalar.activation(
                            p_un,
                            a_t,
                            mybir.ActivationFunctionType.Exp,
                            accum_out=rsum,
                        )
                        rinv = small_pool.tile([P, 1], f32, tag="rinv")
                        nc.vector.reciprocal(rinv, rsum)

                        p_out = pn_pool.tile([P, S], f32, tag="p_out")
                        nc.vector.tensor_scalar_mul(p_out, p_un, rinv)
                        nc.sync.dma_start(out=outp[h, qs : qs + P, :], in_=p_out)

                        # transpose a into the corner-turn buffer (bf16)
                        for kq in range(2):
                            pt = ps_tr.tile([P, 1024], bf16, tag="ps_tr")
                            for kk in range(8):
                                kj = kq * 8 + kk
                                nc.tensor.transpose(
                                    pt[:, kk * P : (kk + 1) * P],
                                    a_t[:, kj * P : (kj + 1) * P],
                                    ident_bf,
                                )
                            nc.scalar.copy(
                                at_g[:, kq * 8 : kq * 8 + 8, gq * P : (gq + 1) * P],
                                pt.rearrange("p (a b) -> p a b", a=8),
                            )

                    # ---- AV for this group: attn_vT[64, GRP*P] ----
                    av = ps_av.tile([D, GRP * P], f32, tag="av")
                    for kj in range(KC):
                        nc.tensor.matmul(
                            av,
                            v_bf[:, kj, :],
                            at_g[:, kj, :],
                            start=(kj == 0),
                            stop=(kj == KC - 1),
                        )
                    av_sb = vo_pool.tile([D, GRP * P], bf16, tag="av_sb")
                    nc.vector.tensor_copy(av_sb, av)
                    # transpose back to [q, D] and write out
                    pt2 = ps_tr.tile([P, 1024], bf16, tag="ps_tr")
                    for gq in range(GRP):
                        nc.tensor.transpose(
                            pt2[:, gq * 256 : gq * 256 + D],
                            av_sb[:, gq * P : (gq + 1) * P],
                            ident_bf[:D, :D],
                        )
                    vo_sb = vo_pool.tile([P, GRP, D], f32, tag="vo_sb")
                    nc.vector.tensor_copy(
                        vo_sb,
                        pt2.rearrange("p (g c) -> p g c", g=4)[:, :, :D],
                    )
                    qs = g * GRP * P
                    nc.sync.dma_start(
                        out=outv[h, qs : qs + GRP * P, :].rearrange(
                            "(g p) d -> p g d", p=P
                        ),
                        in_=vo_sb,
                    )

    legalize_waits(nc)
    return nc


def kernel(Q, K, V, attn_mask):
    Q = np.ascontiguousarray(np.asarray(Q), dtype=np.float32)
    K = np.ascontiguousarray(np.asarray(K), dtype=np.float32)
    V = np.ascontiguousarray(np.asarray(V), dtype=np.float32)
    M = np.asarray(attn_mask).astype(np.uint8)

    qf = Q.reshape(B * H, S, D)
    kf = K.reshape(B * H, S, D)
    vf = V.reshape(B * H, S, D)
    mf = M.reshape(B * H, S, S)

    if "nc" not in _COMPILED:
        _COMPILED["nc"] = build_bass()
    nc = _COMPILED["nc"]

    in_maps = []
    for c in range(N_CORES):
        sl = slice(c * HEADS_PER_CORE, (c + 1) * HEADS_PER_CORE)
        in_maps.append(
            {
                "q": np.ascontiguousarray(qf[sl]),
                "k": np.ascontiguousarray(kf[sl]),
                "v": np.ascontiguousarray(vf[sl]),
                "m": np.ascontiguousarray(mf[sl]),
            }
        )

    res = run_bass_kernel_spmd(nc, in_maps, core_ids=list(range(N_CORES)))
    results = res.results

    attn_v = np.concatenate([results[c]["out_v"] for c in range(N_CORES)], axis=0)
    attn_p = np.concatenate([results[c]["out_p"] for c in range(N_CORES)], axis=0)
    attn_v = attn_v.reshape(B, H, S, D).astype(np.float32)
    attn_p = attn_p.reshape(B, H, S, S).astype(np.float32)
    return attn_v, attn_p


if __name__ == "__main__":
    rng = np.random.default_rng(0)
    Q = rng.standard_normal((B, H, S, D), dtype=np.float32)
    K = rng.standard_normal((B, H, S, D), dtype=np.float32)
    V = rng.standard_normal((B, H, S, D), dtype=np.float32)
    Mm = rng.integers(0, 2, size=(B, H, S, S)).astype(bool)
    av, ap = kernel(Q, K, V, Mm)
    print(av.shape, ap.shape, av.dtype, ap.dtype)


# revision 15
# speedup vs baseline: 5.2141x; 1.0285x over previous
"""Trainium2 Bass kernel for masked attention with pre-softmax-score AV matmul.

Reference semantics (faithful to the source module's bug):
    a = (Q @ K^T) / sqrt(D);  a = where(mask, -1e9, a)
    attn_p = softmax(a, axis=-1)
    attn_v = a @ V            # uses pre-softmax masked scores, NOT attn_p
    returns (attn_v, attn_p)

Shapes: Q,K,V (2,8,2048,64) f32; mask (2,8,2048,2048) bool.
Sharding: B*H = 16 head-slices, 2 per core across 8 cores (pure data
parallel, no collectives).
"""

import sys

sys.path.insert(0, "/opt/trn_rl_repo")

import ml_dtypes
import numpy as np

import concourse.bass as bass
import concourse.tile as tile
from concourse import mybir
from concourse.bass_utils import run_bass_kernel_spmd
from concourse.masks import make_identity

B, H, S, D = 2, 8, 2048, 64
N_CORES = 8
HEADS_PER_CORE = (B * H) // N_CORES  # 2
P = 128                              # partition tile (q rows per tile)
NQT = S // P                         # 16 q-tiles per head
KC = S // P                          # 16 k-chunks of 128
GRP = 4                              # q-tiles per AV matmul group
SCALE = 1.0 / np.sqrt(np.float32(D))  # 0.125, exact in fp32
NEG = -1.0e9

f32 = mybir.dt.float32
f32r = mybir.dt.float32r
bf16 = mybir.dt.bfloat16
u8 = mybir.dt.uint8

_COMPILED = {}


def legalize_waits(nc):
    """This walrus build allows only ONE sync-wait command per instruction.

    Tile's wait assigner can attach several (one per upstream engine), which
    fails codegen with 'Too many sync wait commands'. Hoist all but the last
    wait onto preceding same-engine NoOps (program order on the engine's
    sequencer makes this semantically identical).
    """
    n_fixed = 0
    for fn in nc.m.functions:
        for blk in fn.blocks:
            insts = list(blk.instructions)
            new_list = []
            changed = False
            for inst in insts:
                si = inst.sync_info
                if si is not None and si.on_wait is not None and len(si.on_wait) > 1:
                    waits = list(si.on_wait)
                    for wi, w in enumerate(waits[:-1]):
                        new_list.append(
                            mybir.InstNoOp(
                                name=f"{inst.name}-wn{wi}",
                                engine=inst.engine,
                                sync_info=mybir.SyncInfo(on_wait=[w], on_update=[]),
                            )
                        )
                    inst.sync_info = mybir.SyncInfo(
                        on_wait=waits[-1:], on_update=list(si.on_update or [])
                    )
                    changed = True
                    n_fixed += 1
                new_list.append(inst)
            if changed:
                blk.instructions = new_list
    return n_fixed


def build_bass():
    nc = bass.Bass()

    q_ext = nc.declare_dram_parameter("q", [HEADS_PER_CORE, S, D], bf16, isOutput=False)
    k_ext = nc.declare_dram_parameter("k", [HEADS_PER_CORE, S, D], bf16, isOutput=False)
    v_ext = nc.declare_dram_parameter("v", [HEADS_PER_CORE, S, D], bf16, isOutput=False)
    m_ext = nc.declare_dram_parameter("m", [HEADS_PER_CORE, S, S], u8, isOutput=False)
    outv = nc.declare_dram_parameter("out_v", [HEADS_PER_CORE, S, D], f32, isOutput=True)
    outp = nc.declare_dram_parameter("out_p", [HEADS_PER_CORE, S, S], f32, isOutput=True)

    with tile.TileContext(nc) as tc:
        with (
            tc.tile_pool(name="const", bufs=1) as const_pool,
            tc.tile_pool(name="head", bufs=2) as head_pool,
            tc.tile_pool(name="mask", bufs=4) as mask_pool,
            tc.tile_pool(name="sc", bufs=3) as sc_pool,
            tc.tile_pool(name="pn", bufs=3) as pn_pool,
            tc.tile_pool(name="at", bufs=2) as at_pool,
            tc.tile_pool(name="small", bufs=4) as small_pool,
            tc.tile_pool(name="vo", bufs=4) as vo_pool,
            tc.tile_pool(name="ps_qk", bufs=2, space="PSUM") as ps_qk,
            tc.tile_pool(name="ps_tr", bufs=2, space="PSUM") as ps_tr,
            tc.tile_pool(name="ps_av", bufs=2, space="PSUM") as ps_av,
        ):
            ident = const_pool.tile([P, P], f32)
            make_identity(nc, ident)
            ident_bf = const_pool.tile([P, P], bf16)
            nc.vector.tensor_copy(ident_bf, ident)

            for h in range(HEADS_PER_CORE):
                # ---- per-head prep: Q/K/V arrive bf16 (Q pre-scaled on host).
                # QT/KT are built duplicated across both partition halves so QK
                # matmuls can row-pack; a stride-0 doubled AP makes one PE
                # transpose emit both halves at once. ----
                q_bf = head_pool.tile([P, KC, D], bf16, tag="q_bf")
                k_bf = head_pool.tile([P, KC, D], bf16, tag="k_bf")
                v_bf = head_pool.tile([P, KC, D], bf16, tag="v_bf")
                nc.sync.dma_start(
                    out=q_bf, in_=q_ext[h].rearrange("(t p) d -> p t d", p=P)
                )
                nc.sync.dma_start(
                    out=k_bf, in_=k_ext[h].rearrange("(t p) d -> p t d", p=P)
                )
                nc.sync.dma_start(
                    out=v_bf, in_=v_ext[h].rearrange("(t p) d -> p t d", p=P)
                )

                qt = head_pool.tile([2 * D, S], bf16, tag="qt")
                kt = head_pool.tile([2 * D, S], bf16, tag="kt")

                for t in range(0, KC, 4):
                    pt = ps_tr.tile([P, 1024], bf16, tag="ps_tr")
                    for c in range(4):
                        nc.tensor.transpose(
                            pt[:D, c * P : (c + 1) * P], q_bf[:, t + c, :], ident_bf
                        )
                        nc.tensor.transpose(
                            pt[:D, 512 + c * P : 512 + (c + 1) * P],
                            k_bf[:, t + c, :],
                            ident_bf,
                        )
                    nc.vector.tensor_copy(qt[:D, t * P : (t + 4) * P], pt[:D, 0:512])
                    nc.vector.tensor_copy(qt[D:, t * P : (t + 4) * P], pt[:D, 0:512])
                    nc.vector.tensor_copy(kt[:D, t * P : (t + 4) * P], pt[:D, 512:1024])
                    nc.vector.tensor_copy(kt[D:, t * P : (t + 4) * P], pt[:D, 512:1024])

                for g in range(NQT // GRP):
                    # corner-turn buffer: aT_g[p, kc, q_in_group]
                    at_g = at_pool.tile([P, KC, GRP * P], bf16, tag="at_g")

                    for gq in range(GRP):
                        qi = g * GRP + gq
                        qs = qi * P

                        m_tile = mask_pool.tile([P, S], u8, tag="m")
                        nc.sync.dma_start(out=m_tile, in_=m_ext[h, qs : qs + P, :])

                        # a = qk + mask * (-1e9)  (masked scores, bf16),
                        # in two pipelined halves so PE can start the next
                        # half/tile while DVE drains the previous one
                        a_t = sc_pool.tile([P, S], bf16, tag="a")
                        for half in range(2):
                            hs = half * 1024
                            qk = ps_qk.tile([P, 1024], f32, tag="qk")
                            nc.tensor.matmul(
                                qk[:, 0:512],
                                qt[:D, qs : qs + P],
                                kt[:D, hs : hs + 512],
                                start=True,
                                stop=True,
                            )
                            nc.tensor.matmul(
                                qk[:, 512:1024],
                                qt[D:, qs : qs + P],
                                kt[D:, hs + 512 : hs + 1024],
                                start=True,
                                stop=True,
                            )
                            nc.vector.scalar_tensor_tensor(
                                out=a_t[:, hs : hs + 1024],
                                in0=m_tile[:, hs : hs + 1024],
                                scalar=NEG,
                                in1=qk,
                                op0=mybir.AluOpType.mult,
                                op1=mybir.AluOpType.add,
                            )

                        # p_unnorm = exp(a), rowsum via accum
                        p_un = pn_pool.tile([P, S], f32, tag="p_un")
                        rsum = small_pool.tile([P, 1], f32, tag="rsum")
                        nc.scalar.activation(
                            p_un,
                            a_t,
                            mybir.ActivationFunctionType.Exp,
                            accum_out=rsum,
                        )
                        rinv = small_pool.tile([P, 1], f32, tag="rinv")
                        nc.vector.reciprocal(rinv, rsum)

                        p_out = pn_pool.tile([P, S], f32, tag="p_out")
                        nc.vector.tensor_scalar_mul(p_out, p_un, rinv)
                        nc.sync.dma_start(out=outp[h, qs : qs + P, :], in_=p_out)

                        # transpose a into the corner-turn buffer (bf16)
                        for kq in range(2):
                            pt = ps_tr.tile([P, 1024], bf16, tag="ps_tr")
                            for kk in range(8):
                                kj = kq * 8 + kk
                                nc.tensor.transpose(
                                    pt[:, kk * P : (kk + 1) * P],
                                    a_t[:, kj * P : (kj + 1) * P],
                                    ident_bf,
                                )
                            nc.scalar.copy(
                                at_g[:, kq * 8 : kq * 8 + 8, gq * P : (gq + 1) * P],
                                pt.rearrange("p (a b) -> p a b", a=8),
                            )

                    # ---- AV for this group: attn_vT[64, GRP*P] ----
                    av = ps_av.tile([D, GRP * P], f32, tag="av")
                    for kj in range(KC):
                        nc.tensor.matmul(
                            av,
                            v_bf[:, kj, :],
                            at_g[:, kj, :],
                            start=(kj == 0),
                            stop=(kj == KC - 1),
                        )
                    av_sb = vo_pool.tile([D, GRP * P], bf16, tag="av_sb")
                    nc.vector.tensor_copy(av_sb, av)
                    # transpose back to [q, D] and write out
                    pt2 = ps_tr.tile([P, 1024], bf16, tag="ps_tr")
                    for gq in range(GRP):
                        nc.tensor.transpose(
                            pt2[:, gq * 256 : gq * 256 + D],
                            av_sb[:, gq * P : (gq + 1) * P],
                            ident_bf[:D, :D],
                        )
                    vo_sb = vo_pool.tile([P, GRP, D], f32, tag="vo_sb")
                    nc.vector.tensor_copy(
                        vo_sb,
                        pt2.rearrange("p (g c) -> p g c", g=4)[:, :, :D],
                    )
                    qs = g * GRP * P
                    nc.sync.dma_start(
                        out=outv[h, qs : qs + GRP * P, :].rearrange(
                            "(g p) d -> p g d", p=P
                        ),
                        in_=vo_sb,
                    )

    legalize_waits(nc)
    return nc


def kernel(Q, K, V, attn_mask):
    bf = ml_dtypes.bfloat16
    Q = np.ascontiguousarray(np.asarray(Q), dtype=np.float32)
    K = np.ascontiguousarray(np.asarray(K), dtype=np.float32)
    V = np.ascontiguousarray(np.asarray(V), dtype=np.float32)
    M = np.asarray(attn_mask).astype(np.uint8)

    # fold the 1/sqrt(D) scale into Q on the host; ship operands as bf16
    qf = (Q.reshape(B * H, S, D) * np.float32(SCALE)).astype(bf)
    kf = K.reshape(B * H, S, D).astype(bf)
    vf = V.reshape(B * H, S, D).astype(bf)
    mf = M.reshape(B * H, S, S)

    if "nc" not in _COMPILED:
        _COMPILED["nc"] = build_bass()
    nc = _COMPILED["nc"]

    in_maps = []
    for c in range(N_CORES):
        sl = slice(c * HEADS_PER_CORE, (c + 1) * HEADS_PER_CORE)
        in_maps.append(
            {
                "q": np.ascontiguousarray(qf[sl]),
                "k": np.ascontiguousarray(kf[sl]),
                "v": np.ascontiguousarray(vf[sl]),
                "m": np.ascontiguousarray(mf[sl]),
            }
        )

    res = run_bass_kernel_spmd(nc, in_maps, core_ids=list(range(N_CORES)))
    results = res.results

    attn_v = np.concatenate([results[c]["out_v"] for c in range(N_CORES)], axis=0)
    attn_p = np.concatenate([results[c]["out_p"] for c in range(N_CORES)], axis=0)
    attn_v = attn_v.reshape(B, H, S, D).astype(np.float32)
    attn_p = attn_p.reshape(B, H, S, S).astype(np.float32)
    return attn_v, attn_p


if __name__ == "__main__":
    rng = np.random.default_rng(0)
    Q = rng.standard_normal((B, H, S, D), dtype=np.float32)
    K = rng.standard_normal((B, H, S, D), dtype=np.float32)
    V = rng.standard_normal((B, H, S, D), dtype=np.float32)
    Mm = rng.integers(0, 2, size=(B, H, S, S)).astype(bool)
    av, ap = kernel(Q, K, V, Mm)
    print(av.shape, ap.shape, av.dtype, ap.dtype)


# revision 16
# speedup vs baseline: 5.3542x; 1.0269x over previous
"""Trainium2 Bass kernel for masked attention with pre-softmax-score AV matmul.

Reference semantics (faithful to the source module's bug):
    a = (Q @ K^T) / sqrt(D);  a = where(mask, -1e9, a)
    attn_p = softmax(a, axis=-1)
    attn_v = a @ V            # uses pre-softmax masked scores, NOT attn_p
    returns (attn_v, attn_p)

Shapes: Q,K,V (2,8,2048,64) f32; mask (2,8,2048,2048) bool.
Sharding: B*H = 16 head-slices, 2 per core across 8 cores (pure data
parallel, no collectives).
"""

import sys

sys.path.insert(0, "/opt/trn_rl_repo")

import ml_dtypes
import numpy as np

import concourse.bass as bass
import concourse.tile as tile
from concourse import mybir
from concourse.bass_utils import run_bass_kernel_spmd
from concourse.masks import make_identity

B, H, S, D = 2, 8, 2048, 64
N_CORES = 8
HEADS_PER_CORE = (B * H) // N_CORES  # 2
P = 128                              # partition tile (q rows per tile)
NQT = S // P                         # 16 q-tiles per head
KC = S // P                          # 16 k-chunks of 128
GRP = 4                              # q-tiles per AV matmul group
SCALE = 1.0 / np.sqrt(np.float32(D))  # 0.125, exact in fp32
NEG = -1.0e9

f32 = mybir.dt.float32
f32r = mybir.dt.float32r
bf16 = mybir.dt.bfloat16
u8 = mybir.dt.uint8

_COMPILED = {}


def legalize_waits(nc):
    """This walrus build allows only ONE sync-wait command per instruction.

    Tile's wait assigner can attach several (one per upstream engine), which
    fails codegen with 'Too many sync wait commands'. Hoist all but the last
    wait onto preceding same-engine NoOps (program order on the engine's
    sequencer makes this semantically identical).
    """
    n_fixed = 0
    for fn in nc.m.functions:
        for blk in fn.blocks:
            insts = list(blk.instructions)
            new_list = []
            changed = False
            for inst in insts:
                si = inst.sync_info
                if si is not None and si.on_wait is not None and len(si.on_wait) > 1:
                    waits = list(si.on_wait)
                    for wi, w in enumerate(waits[:-1]):
                        new_list.append(
                            mybir.InstNoOp(
                                name=f"{inst.name}-wn{wi}",
                                engine=inst.engine,
                                sync_info=mybir.SyncInfo(on_wait=[w], on_update=[]),
                            )
                        )
                    inst.sync_info = mybir.SyncInfo(
                        on_wait=waits[-1:], on_update=list(si.on_update or [])
                    )
                    changed = True
                    n_fixed += 1
                new_list.append(inst)
            if changed:
                blk.instructions = new_list
    return n_fixed


def build_bass():
    nc = bass.Bass()

    q_ext = nc.declare_dram_parameter("q", [HEADS_PER_CORE, S, D], bf16, isOutput=False)
    k_ext = nc.declare_dram_parameter("k", [HEADS_PER_CORE, S, D], bf16, isOutput=False)
    v_ext = nc.declare_dram_parameter("v", [HEADS_PER_CORE, S, D], bf16, isOutput=False)
    m_ext = nc.declare_dram_parameter("m", [HEADS_PER_CORE, S, S], u8, isOutput=False)
    outv = nc.declare_dram_parameter("out_v", [HEADS_PER_CORE, S, D], f32, isOutput=True)
    outp = nc.declare_dram_parameter("out_p", [HEADS_PER_CORE, S, S], f32, isOutput=True)

    with tile.TileContext(nc) as tc:
        with (
            tc.tile_pool(name="const", bufs=1) as const_pool,
            tc.tile_pool(name="head", bufs=2) as head_pool,
            tc.tile_pool(name="mask", bufs=4) as mask_pool,
            tc.tile_pool(name="sc", bufs=3) as sc_pool,
            tc.tile_pool(name="pn", bufs=3) as pn_pool,
            tc.tile_pool(name="at", bufs=2) as at_pool,
            tc.tile_pool(name="small", bufs=4) as small_pool,
            tc.tile_pool(name="vo", bufs=4) as vo_pool,
            tc.tile_pool(name="ps_qk", bufs=2, space="PSUM") as ps_qk,
            tc.tile_pool(name="ps_tr", bufs=2, space="PSUM") as ps_tr,
            tc.tile_pool(name="ps_av", bufs=2, space="PSUM") as ps_av,
        ):
            ident = const_pool.tile([P, P], f32)
            make_identity(nc, ident)
            ident_bf = const_pool.tile([P, P], bf16)
            nc.vector.tensor_copy(ident_bf, ident)

            for h in range(HEADS_PER_CORE):
                # ---- per-head prep: Q/K/V arrive bf16 (Q pre-scaled on host).
                # QT/KT are built duplicated across both partition halves so QK
                # matmuls can row-pack; a stride-0 doubled AP makes one PE
                # transpose emit both halves at once. ----
                q_bf = head_pool.tile([P, KC, D], bf16, tag="q_bf")
                k_bf = head_pool.tile([P, KC, D], bf16, tag="k_bf")
                v_bf = head_pool.tile([P, KC, D], bf16, tag="v_bf")
                nc.sync.dma_start(
                    out=q_bf, in_=q_ext[h].rearrange("(t p) d -> p t d", p=P)
                )
                nc.sync.dma_start(
                    out=k_bf, in_=k_ext[h].rearrange("(t p) d -> p t d", p=P)
                )
                nc.sync.dma_start(
                    out=v_bf, in_=v_ext[h].rearrange("(t p) d -> p t d", p=P)
                )

                qt = head_pool.tile([2 * D, S], bf16, tag="qt")
                kt = head_pool.tile([2 * D, S], bf16, tag="kt")

                for t in range(0, KC, 4):
                    pt = ps_tr.tile([P, 1024], bf16, tag="ps_tr")
                    for c in range(4):
                        nc.tensor.transpose(
                            pt[:D, c * P : (c + 1) * P], q_bf[:, t + c, :], ident_bf
                        )
                        nc.tensor.transpose(
                            pt[:D, 512 + c * P : 512 + (c + 1) * P],
                            k_bf[:, t + c, :],
                            ident_bf,
                        )
                    nc.vector.tensor_copy(qt[:D, t * P : (t + 4) * P], pt[:D, 0:512])
                    nc.vector.tensor_copy(qt[D:, t * P : (t + 4) * P], pt[:D, 0:512])
                    nc.vector.tensor_copy(kt[:D, t * P : (t + 4) * P], pt[:D, 512:1024])
                    nc.vector.tensor_copy(kt[D:, t * P : (t + 4) * P], pt[:D, 512:1024])

                for g in range(NQT // GRP):
                    # corner-turn buffer: aT_g[p, kc, q_in_group]
                    at_g = at_pool.tile([P, KC, GRP * P], bf16, tag="at_g")

                    for gq in range(GRP):
                        qi = g * GRP + gq
                        qs = qi * P

                        m_tile = mask_pool.tile([P, S], u8, tag="m")
                        nc.sync.dma_start(out=m_tile, in_=m_ext[h, qs : qs + P, :])

                        # a = qk + mask * (-1e9)  (masked scores, bf16),
                        # in two pipelined halves so PE can start the next
                        # half/tile while DVE drains the previous one
                        a_t = sc_pool.tile([P, S], bf16, tag="a")
                        for half in range(2):
                            hs = half * 1024
                            qk = ps_qk.tile([P, 1024], f32, tag="qk")
                            nc.tensor.matmul(
                                qk[:, 0:512],
                                qt[:D, qs : qs + P],
                                kt[:D, hs : hs + 512],
                                start=True,
                                stop=True,
                            )
                            nc.tensor.matmul(
                                qk[:, 512:1024],
                                qt[D:, qs : qs + P],
                                kt[D:, hs + 512 : hs + 1024],
                                start=True,
                                stop=True,
                            )
                            nc.vector.scalar_tensor_tensor(
                                out=a_t[:, hs : hs + 1024],
                                in0=m_tile[:, hs : hs + 1024],
                                scalar=NEG,
                                in1=qk,
                                op0=mybir.AluOpType.mult,
                                op1=mybir.AluOpType.add,
                            )

                        # p_unnorm = exp(a), rowsum via accum
                        p_un = pn_pool.tile([P, S], bf16, tag="p_un")
                        rsum = small_pool.tile([P, 1], f32, tag="rsum")
                        nc.scalar.activation(
                            p_un,
                            a_t,
                            mybir.ActivationFunctionType.Exp,
                            accum_out=rsum,
                        )
                        rinv = small_pool.tile([P, 1], f32, tag="rinv")
                        nc.vector.reciprocal(rinv, rsum)

                        p_out = pn_pool.tile([P, S], bf16, tag="p_out")
                        nc.vector.tensor_scalar_mul(p_out, p_un, rinv)
                        # SWDGE casts bf16 -> f32 during the store
                        nc.gpsimd.dma_start(out=outp[h, qs : qs + P, :], in_=p_out)

                        # transpose a into the corner-turn buffer (bf16)
                        for kq in range(2):
                            pt = ps_tr.tile([P, 1024], bf16, tag="ps_tr")
                            for kk in range(8):
                                kj = kq * 8 + kk
                                nc.tensor.transpose(
                                    pt[:, kk * P : (kk + 1) * P],
                                    a_t[:, kj * P : (kj + 1) * P],
                                    ident_bf,
                                )
                            copy_eng = nc.scalar if kq == 0 else nc.vector
                            if kq == 0:
                                nc.scalar.copy(
                                    at_g[:, 0:8, gq * P : (gq + 1) * P],
                                    pt.rearrange("p (a b) -> p a b", a=8),
                                )
                            else:
                                nc.vector.tensor_copy(
                                    at_g[:, 8:16, gq * P : (gq + 1) * P],
                                    pt.rearrange("p (a b) -> p a b", a=8),
                                )

                    # ---- AV for this group: attn_vT[64, GRP*P] ----
                    av = ps_av.tile([D, GRP * P], f32, tag="av")
                    for kj in range(KC):
                        nc.tensor.matmul(
                            av,
                            v_bf[:, kj, :],
                            at_g[:, kj, :],
                            start=(kj == 0),
                            stop=(kj == KC - 1),
                        )
                    av_sb = vo_pool.tile([D, GRP * P], bf16, tag="av_sb")
                    nc.vector.tensor_copy(av_sb, av)
                    # transpose back to [q, D] and write out
                    pt2 = ps_tr.tile([P, 1024], bf16, tag="ps_tr")
                    for gq in range(GRP):
                        nc.tensor.transpose(
                            pt2[:, gq * 256 : gq * 256 + D],
                            av_sb[:, gq * P : (gq + 1) * P],
                            ident_bf[:D, :D],
                        )
                    vo_sb = vo_pool.tile([P, GRP, D], f32, tag="vo_sb")
                    nc.vector.tensor_copy(
                        vo_sb,
                        pt2.rearrange("p (g c) -> p g c", g=4)[:, :, :D],
                    )
                    qs = g * GRP * P
                    nc.sync.dma_start(
                        out=outv[h, qs : qs + GRP * P, :].rearrange(
                            "(g p) d -> p g d", p=P
                        ),
                        in_=vo_sb,
                    )

    legalize_waits(nc)
    return nc


def kernel(Q, K, V, attn_mask):
    bf = ml_dtypes.bfloat16
    Q = np.ascontiguousarray(np.asarray(Q), dtype=np.float32)
    K = np.ascontiguousarray(np.asarray(K), dtype=np.float32)
    V = np.ascontiguousarray(np.asarray(V), dtype=np.float32)
    M = np.asarray(attn_mask).astype(np.uint8)

    # fold the 1/sqrt(D) scale into Q on the host; ship operands as bf16
    qf = (Q.reshape(B * H, S, D) * np.float32(SCALE)).astype(bf)
    kf = K.reshape(B * H, S, D).astype(bf)
    vf = V.reshape(B * H, S, D).astype(bf)
    mf = M.reshape(B * H, S, S)

    if "nc" not in _COMPILED:
        _COMPILED["nc"] = build_bass()
    nc = _COMPILED["nc"]

    in_maps = []
    for c in range(N_CORES):
        sl = slice(c * HEADS_PER_CORE, (c + 1) * HEADS_PER_CORE)
        in_maps.append(
            {
                "q": np.ascontiguousarray(qf[sl]),
                "k": np.ascontiguousarray(kf[sl]),
                "v": np.ascontiguousarray(vf[sl]),
                "m": np.ascontiguousarray(mf[sl]),
            }
        )

    res = run_bass_kernel_spmd(nc, in_maps, core_ids=list(range(N_CORES)))
    results = res.results

    attn_v = np.concatenate([results[c]["out_v"] for c in range(N_CORES)], axis=0)
    attn_p = np.concatenate([results[c]["out_p"] for c in range(N_CORES)], axis=0)
    attn_v = attn_v.reshape(B, H, S, D).astype(np.float32)
    attn_p = attn_p.reshape(B, H, S, S).astype(np.float32)
    return attn_v, attn_p


if __name__ == "__main__":
    rng = np.random.default_rng(0)
    Q = rng.standard_normal((B, H, S, D), dtype=np.float32)
    K = rng.standard_normal((B, H, S, D), dtype=np.float32)
    V = rng.standard_normal((B, H, S, D), dtype=np.float32)
    Mm = rng.integers(0, 2, size=(B, H, S, S)).astype(bool)
    av, ap = kernel(Q, K, V, Mm)
    print(av.shape, ap.shape, av.dtype, ap.dtype)


# revision 17
# speedup vs baseline: 5.4183x; 1.0120x over previous
"""Trainium2 Bass kernel for masked attention with pre-softmax-score AV matmul.

Reference semantics (faithful to the source module's bug):
    a = (Q @ K^T) / sqrt(D);  a = where(mask, -1e9, a)
    attn_p = softmax(a, axis=-1)
    attn_v = a @ V            # uses pre-softmax masked scores, NOT attn_p
    returns (attn_v, attn_p)

Shapes: Q,K,V (2,8,2048,64) f32; mask (2,8,2048,2048) bool.
Sharding: B*H = 16 head-slices, 2 per core across 8 cores (pure data
parallel, no collectives).
"""

import sys

sys.path.insert(0, "/opt/trn_rl_repo")

import ml_dtypes
import numpy as np

import concourse.bass as bass
import concourse.tile as tile
from concourse import mybir
from concourse.bass_utils import run_bass_kernel_spmd
from concourse.masks import make_identity

B, H, S, D = 2, 8, 2048, 64
N_CORES = 8
HEADS_PER_CORE = (B * H) // N_CORES  # 2
P = 128                              # partition tile (q rows per tile)
NQT = S // P                         # 16 q-tiles per head
KC = S // P                          # 16 k-chunks of 128
GRP = 4                              # q-tiles per AV matmul group
SCALE = 1.0 / np.sqrt(np.float32(D))  # 0.125, exact in fp32
NEG = -1.0e9

f32 = mybir.dt.float32
f32r = mybir.dt.float32r
bf16 = mybir.dt.bfloat16
u8 = mybir.dt.uint8

_COMPILED = {}


def legalize_waits(nc):
    """This walrus build allows only ONE sync-wait command per instruction.

    Tile's wait assigner can attach several (one per upstream engine), which
    fails codegen with 'Too many sync wait commands'. Hoist all but the last
    wait onto preceding same-engine NoOps (program order on the engine's
    sequencer makes this semantically identical).
    """
    n_fixed = 0
    for fn in nc.m.functions:
        for blk in fn.blocks:
            insts = list(blk.instructions)
            new_list = []
            changed = False
            for inst in insts:
                si = inst.sync_info
                if si is not None and si.on_wait is not None and len(si.on_wait) > 1:
                    waits = list(si.on_wait)
                    for wi, w in enumerate(waits[:-1]):
                        new_list.append(
                            mybir.InstNoOp(
                                name=f"{inst.name}-wn{wi}",
                                engine=inst.engine,
                                sync_info=mybir.SyncInfo(on_wait=[w], on_update=[]),
                            )
                        )
                    inst.sync_info = mybir.SyncInfo(
                        on_wait=waits[-1:], on_update=list(si.on_update or [])
                    )
                    changed = True
                    n_fixed += 1
                new_list.append(inst)
            if changed:
                blk.instructions = new_list
    return n_fixed


def build_bass():
    nc = bass.Bass()

    q_ext = nc.declare_dram_parameter("q", [HEADS_PER_CORE, S, D], bf16, isOutput=False)
    k_ext = nc.declare_dram_parameter("k", [HEADS_PER_CORE, S, D], bf16, isOutput=False)
    v_ext = nc.declare_dram_parameter("v", [HEADS_PER_CORE, S, D], bf16, isOutput=False)
    m_ext = nc.declare_dram_parameter("m", [HEADS_PER_CORE, S, S], u8, isOutput=False)
    outv = nc.declare_dram_parameter("out_v", [HEADS_PER_CORE, S, D], f32, isOutput=True)
    outp = nc.declare_dram_parameter("out_p", [HEADS_PER_CORE, S, S], f32, isOutput=True)

    with tile.TileContext(nc) as tc:
        with (
            tc.tile_pool(name="const", bufs=1) as const_pool,
            tc.tile_pool(name="head", bufs=2) as head_pool,
            tc.tile_pool(name="mask", bufs=4) as mask_pool,
            tc.tile_pool(name="sc", bufs=3) as sc_pool,
            tc.tile_pool(name="pn", bufs=3) as pn_pool,
            tc.tile_pool(name="at", bufs=2) as at_pool,
            tc.tile_pool(name="small", bufs=4) as small_pool,
            tc.tile_pool(name="vo", bufs=4) as vo_pool,
            tc.tile_pool(name="ps_qk", bufs=2, space="PSUM") as ps_qk,
            tc.tile_pool(name="ps_tr", bufs=2, space="PSUM") as ps_tr,
            tc.tile_pool(name="ps_av", bufs=2, space="PSUM") as ps_av,
        ):
            ident = const_pool.tile([P, P], f32)
            make_identity(nc, ident)
            ident_bf = const_pool.tile([P, P], bf16)
            nc.vector.tensor_copy(ident_bf, ident)

            for h in range(HEADS_PER_CORE):
                # ---- per-head prep: Q/K/V arrive bf16 (Q pre-scaled on host).
                # QT/KT are built duplicated across both partition halves so QK
                # matmuls can row-pack; a stride-0 doubled AP makes one PE
                # transpose emit both halves at once. ----
                q_bf = head_pool.tile([P, KC, D], bf16, tag="q_bf")
                k_bf = head_pool.tile([P, KC, D], bf16, tag="k_bf")
                v_bf = head_pool.tile([P, KC, D], bf16, tag="v_bf")
                nc.sync.dma_start(
                    out=q_bf, in_=q_ext[h].rearrange("(t p) d -> p t d", p=P)
                )
                nc.sync.dma_start(
                    out=k_bf, in_=k_ext[h].rearrange("(t p) d -> p t d", p=P)
                )
                nc.sync.dma_start(
                    out=v_bf, in_=v_ext[h].rearrange("(t p) d -> p t d", p=P)
                )

                qt = head_pool.tile([2 * D, S], bf16, tag="qt")
                kt = head_pool.tile([2 * D, S], bf16, tag="kt")

                for t in range(0, KC, 4):
                    pt = ps_tr.tile([P, 1024], bf16, tag="ps_tr")
                    for c in range(4):
                        nc.tensor.transpose(
                            pt[:D, c * P : (c + 1) * P], q_bf[:, t + c, :], ident_bf
                        )
                        nc.tensor.transpose(
                            pt[:D, 512 + c * P : 512 + (c + 1) * P],
                            k_bf[:, t + c, :],
                            ident_bf,
                        )
                    nc.vector.tensor_copy(qt[:D, t * P : (t + 4) * P], pt[:D, 0:512])
                    nc.vector.tensor_copy(qt[D:, t * P : (t + 4) * P], pt[:D, 0:512])
                    nc.vector.tensor_copy(kt[:D, t * P : (t + 4) * P], pt[:D, 512:1024])
                    nc.vector.tensor_copy(kt[D:, t * P : (t + 4) * P], pt[:D, 512:1024])

                for g in range(NQT // GRP):
                    # corner-turn buffer: aT_g[p, kc, q_in_group]
                    at_g = at_pool.tile([P, KC, GRP * P], bf16, tag="at_g")

                    for gq in range(GRP):
                        qi = g * GRP + gq
                        qs = qi * P

                        m_tile = mask_pool.tile([P, S], u8, tag="m")
                        nc.sync.dma_start(out=m_tile, in_=m_ext[h, qs : qs + P, :])

                        # a = qk + mask * (-1e9)  (masked scores, bf16),
                        # in two pipelined halves so PE can start the next
                        # half/tile while DVE drains the previous one
                        a_t = sc_pool.tile([P, S], bf16, tag="a")
                        for half in range(2):
                            hs = half * 1024
                            qk = ps_qk.tile([P, 1024], f32, tag="qk")
                            nc.tensor.matmul(
                                qk[:, 0:512],
                                qt[:D, qs : qs + P],
                                kt[:D, hs : hs + 512],
                                start=True,
                                stop=True,
                            )
                            nc.tensor.matmul(
                                qk[:, 512:1024],
                                qt[D:, qs : qs + P],
                                kt[D:, hs + 512 : hs + 1024],
                                start=True,
                                stop=True,
                            )
                            nc.vector.scalar_tensor_tensor(
                                out=a_t[:, hs : hs + 1024],
                                in0=m_tile[:, hs : hs + 1024],
                                scalar=NEG,
                                in1=qk,
                                op0=mybir.AluOpType.mult,
                                op1=mybir.AluOpType.add,
                            )

                        # p_unnorm = exp(a), rowsum via accum
                        p_un = pn_pool.tile([P, S], bf16, tag="p_un")
                        rsum = small_pool.tile([P, 1], f32, tag="rsum")
                        nc.scalar.activation(
                            p_un,
                            a_t,
                            mybir.ActivationFunctionType.Exp,
                            accum_out=rsum,
                        )
                        rinv = small_pool.tile([P, 1], f32, tag="rinv")
                        nc.vector.reciprocal(rinv, rsum)

                        p_out = pn_pool.tile([P, S], bf16, tag="p_out")
                        nc.vector.tensor_scalar_mul(p_out, p_un, rinv)
                        # SWDGE casts bf16 -> f32 during the store
                        nc.gpsimd.dma_start(out=outp[h, qs : qs + P, :], in_=p_out)

                        # transpose a into the corner-turn buffer (bf16)
                        for kq in range(2):
                            pt = ps_tr.tile([P, 1024], bf16, tag="ps_tr")
                            for kk in range(8):
                                kj = kq * 8 + kk
                                nc.tensor.transpose(
                                    pt[:, kk * P : (kk + 1) * P],
                                    a_t[:, kj * P : (kj + 1) * P],
                                    ident_bf,
                                )
                            dst = at_g[
                                :, kq * 8 : kq * 8 + 8, gq * P : (gq + 1) * P
                            ]
                            srcp = pt.rearrange("p (a b) -> p a b", a=8)
                            if kq == 1 and qi % 3 == 2:
                                nc.vector.tensor_copy(dst, srcp)
                            else:
                                nc.scalar.copy(dst, srcp)

                    # ---- AV for this group: attn_vT[64, GRP*P] ----
                    av = ps_av.tile([D, GRP * P], f32, tag="av")
                    for kj in range(KC):
                        nc.tensor.matmul(
                            av,
                            v_bf[:, kj, :],
                            at_g[:, kj, :],
                            start=(kj == 0),
                            stop=(kj == KC - 1),
                        )
                    av_sb = vo_pool.tile([D, GRP * P], bf16, tag="av_sb")
                    nc.vector.tensor_copy(av_sb, av)
                    # transpose back to [q, D] and write out
                    pt2 = ps_tr.tile([P, 1024], bf16, tag="ps_tr")
                    for gq in range(GRP):
                        nc.tensor.transpose(
                            pt2[:, gq * 256 : gq * 256 + D],
                            av_sb[:, gq * P : (gq + 1) * P],
                            ident_bf[:D, :D],
                        )
                    vo_sb = vo_pool.tile([P, GRP, D], f32, tag="vo_sb")
                    nc.vector.tensor_copy(
                        vo_sb,
                        pt2.rearrange("p (g c) -> p g c", g=4)[:, :, :D],
                    )
                    qs = g * GRP * P
                    nc.sync.dma_start(
                        out=outv[h, qs : qs + GRP * P, :].rearrange(
                            "(g p) d -> p g d", p=P
                        ),
                        in_=vo_sb,
                    )

    legalize_waits(nc)
    return nc


def kernel(Q, K, V, attn_mask):
    bf = ml_dtypes.bfloat16
    Q = np.ascontiguousarray(np.asarray(Q), dtype=np.float32)
    K = np.ascontiguousarray(np.asarray(K), dtype=np.float32)
    V = np.ascontiguousarray(np.asarray(V), dtype=np.float32)
    M = np.asarray(attn_mask).astype(np.uint8)

    # fold the 1/sqrt(D) scale into Q on the host; ship operands as bf16
    qf = (Q.reshape(B * H, S, D) * np.float32(SCALE)).astype(bf)
    kf = K.reshape(B * H, S, D).astype(bf)
    vf = V.reshape(B * H, S, D).astype(bf)
    mf = M.reshape(B * H, S, S)

    if "nc" not in _COMPILED:
        _COMPILED["nc"] = build_bass()
    nc = _COMPILED["nc"]

    in_maps = []
    for c in range(N_CORES):
        sl = slice(c * HEADS_PER_CORE, (c + 1) * HEADS_PER_CORE)
        in_maps.append(
            {
                "q": np.ascontiguousarray(qf[sl]),
                "k": np.ascontiguousarray(kf[sl]),
                "v": np.ascontiguousarray(vf[sl]),
                "m": np.ascontiguousarray(mf[sl]),
            }
        )

    res = run_bass_kernel_spmd(nc, in_maps, core_ids=list(range(N_CORES)))
    results = res.results

    attn_v = np.concatenate([results[c]["out_v"] for c in range(N_CORES)], axis=0)
    attn_p = np.concatenate([results[c]["out_p"] for c in range(N_CORES)], axis=0)
    attn_v = attn_v.reshape(B, H, S, D).astype(np.float32)
    attn_p = attn_p.reshape(B, H, S, S).astype(np.float32)
    return attn_v, attn_p


if __name__ == "__main__":
    rng = np.random.default_rng(0)
    Q = rng.standard_normal((B, H, S, D), dtype=np.float32)
    K = rng.standard_normal((B, H, S, D), dtype=np.float32)
    V = rng.standard_normal((B, H, S, D), dtype=np.float32)
    Mm = rng.integers(0, 2, size=(B, H, S, S)).astype(bool)
    av, ap = kernel(Q, K, V, Mm)
    print(av.shape, ap.shape, av.dtype, ap.dtype)


# revision 18
# speedup vs baseline: 5.5870x; 1.0311x over previous
"""Trainium2 Bass kernel for masked attention with pre-softmax-score AV matmul.

Reference semantics (faithful to the source module's bug):
    a = (Q @ K^T) / sqrt(D);  a = where(mask, -1e9, a)
    attn_p = softmax(a, axis=-1)
    attn_v = a @ V            # uses pre-softmax masked scores, NOT attn_p
    returns (attn_v, attn_p)

Shapes: Q,K,V (2,8,2048,64) f32; mask (2,8,2048,2048) bool.
Sharding: B*H = 16 head-slices, 2 per core across 8 cores (pure data
parallel, no collectives).
"""

import sys

sys.path.insert(0, "/opt/trn_rl_repo")

import ml_dtypes
import numpy as np

import concourse.bass as bass
import concourse.tile as tile
from concourse import mybir
from concourse.bass_utils import run_bass_kernel_spmd
from concourse.masks import make_identity

B, H, S, D = 2, 8, 2048, 64
N_CORES = 8
HEADS_PER_CORE = (B * H) // N_CORES  # 2
P = 128                              # partition tile (q rows per tile)
NQT = S // P                         # 16 q-tiles per head
KC = S // P                          # 16 k-chunks of 128
GRP = 4                              # q-tiles per AV matmul group
SCALE = 1.0 / np.sqrt(np.float32(D))  # 0.125, exact in fp32
NEG = -1.0e9

f32 = mybir.dt.float32
f32r = mybir.dt.float32r
bf16 = mybir.dt.bfloat16
u8 = mybir.dt.uint8

_COMPILED = {}


def legalize_waits(nc):
    """This walrus build allows only ONE sync-wait command per instruction.

    Tile's wait assigner can attach several (one per upstream engine), which
    fails codegen with 'Too many sync wait commands'. Hoist all but the last
    wait onto preceding same-engine NoOps (program order on the engine's
    sequencer makes this semantically identical).
    """
    n_fixed = 0
    for fn in nc.m.functions:
        for blk in fn.blocks:
            insts = list(blk.instructions)
            new_list = []
            changed = False
            for inst in insts:
                si = inst.sync_info
                if si is not None and si.on_wait is not None and len(si.on_wait) > 1:
                    waits = list(si.on_wait)
                    for wi, w in enumerate(waits[:-1]):
                        new_list.append(
                            mybir.InstNoOp(
                                name=f"{inst.name}-wn{wi}",
                                engine=inst.engine,
                                sync_info=mybir.SyncInfo(on_wait=[w], on_update=[]),
                            )
                        )
                    inst.sync_info = mybir.SyncInfo(
                        on_wait=waits[-1:], on_update=list(si.on_update or [])
                    )
                    changed = True
                    n_fixed += 1
                new_list.append(inst)
            if changed:
                blk.instructions = new_list
    return n_fixed


def build_bass():
    nc = bass.Bass()

    q_ext = nc.declare_dram_parameter("q", [HEADS_PER_CORE, S, D], bf16, isOutput=False)
    k_ext = nc.declare_dram_parameter("k", [HEADS_PER_CORE, S, D], bf16, isOutput=False)
    v_ext = nc.declare_dram_parameter("v", [HEADS_PER_CORE, S, D], bf16, isOutput=False)
    m_ext = nc.declare_dram_parameter("m", [HEADS_PER_CORE, S, S], u8, isOutput=False)
    outv = nc.declare_dram_parameter("out_v", [HEADS_PER_CORE, S, D], f32, isOutput=True)
    outp = nc.declare_dram_parameter("out_p", [HEADS_PER_CORE, S, S], f32, isOutput=True)

    with tile.TileContext(nc) as tc:
        with (
            tc.tile_pool(name="const", bufs=1) as const_pool,
            tc.tile_pool(name="head", bufs=2) as head_pool,
            tc.tile_pool(name="mask", bufs=8) as mask_pool,
            tc.tile_pool(name="sc", bufs=4) as sc_pool,
            tc.tile_pool(name="pn", bufs=4) as pn_pool,
            tc.tile_pool(name="at", bufs=3) as at_pool,
            tc.tile_pool(name="small", bufs=4) as small_pool,
            tc.tile_pool(name="vo", bufs=6) as vo_pool,
            tc.tile_pool(name="ps_qk", bufs=2, space="PSUM") as ps_qk,
            tc.tile_pool(name="ps_tr", bufs=2, space="PSUM") as ps_tr,
            tc.tile_pool(name="ps_av", bufs=2, space="PSUM") as ps_av,
        ):
            ident = const_pool.tile([P, P], f32)
            make_identity(nc, ident)
            ident_bf = const_pool.tile([P, P], bf16)
            nc.vector.tensor_copy(ident_bf, ident)

            for h in range(HEADS_PER_CORE):
                # ---- per-head prep: Q/K/V arrive bf16 (Q pre-scaled on host).
                # QT/KT are built duplicated across both partition halves so QK
                # matmuls can row-pack; a stride-0 doubled AP makes one PE
                # transpose emit both halves at once. ----
                q_bf = head_pool.tile([P, KC, D], bf16, tag="q_bf")
                k_bf = head_pool.tile([P, KC, D], bf16, tag="k_bf")
                v_bf = head_pool.tile([P, KC, D], bf16, tag="v_bf")
                nc.sync.dma_start(
                    out=q_bf, in_=q_ext[h].rearrange("(t p) d -> p t d", p=P)
                )
                nc.sync.dma_start(
                    out=k_bf, in_=k_ext[h].rearrange("(t p) d -> p t d", p=P)
                )
                nc.sync.dma_start(
                    out=v_bf, in_=v_ext[h].rearrange("(t p) d -> p t d", p=P)
                )

                qt = head_pool.tile([2 * D, S], bf16, tag="qt")
                kt = head_pool.tile([2 * D, S], bf16, tag="kt")

                for t in range(0, KC, 4):
                    pt = ps_tr.tile([P, 1024], bf16, tag="ps_tr")
                    for c in range(4):
                        nc.tensor.transpose(
                            pt[:D, c * P : (c + 1) * P], q_bf[:, t + c, :], ident_bf
                        )
                        nc.tensor.transpose(
                            pt[:D, 512 + c * P : 512 + (c + 1) * P],
                            k_bf[:, t + c, :],
                            ident_bf,
                        )
                    nc.vector.tensor_copy(qt[:D, t * P : (t + 4) * P], pt[:D, 0:512])
                    nc.vector.tensor_copy(qt[D:, t * P : (t + 4) * P], pt[:D, 0:512])
                    nc.vector.tensor_copy(kt[:D, t * P : (t + 4) * P], pt[:D, 512:1024])
                    nc.vector.tensor_copy(kt[D:, t * P : (t + 4) * P], pt[:D, 512:1024])

                for g in range(NQT // GRP):
                    # corner-turn buffer: aT_g[p, kc, q_in_group]
                    at_g = at_pool.tile([P, KC, GRP * P], bf16, tag="at_g")

                    for gq in range(GRP):
                        qi = g * GRP + gq
                        qs = qi * P

                        m_tile = mask_pool.tile([P, S], u8, tag="m")
                        nc.sync.dma_start(out=m_tile, in_=m_ext[h, qs : qs + P, :])

                        # a = qk + mask * (-1e9)  (masked scores, bf16),
                        # in two pipelined halves so PE can start the next
                        # half/tile while DVE drains the previous one
                        a_t = sc_pool.tile([P, S], bf16, tag="a")
                        for half in range(2):
                            hs = half * 1024
                            qk = ps_qk.tile([P, 1024], f32, tag="qk")
                            nc.tensor.matmul(
                                qk[:, 0:512],
                                qt[:D, qs : qs + P],
                                kt[:D, hs : hs + 512],
                                start=True,
                                stop=True,
                            )
                            nc.tensor.matmul(
                                qk[:, 512:1024],
                                qt[D:, qs : qs + P],
                                kt[D:, hs + 512 : hs + 1024],
                                start=True,
                                stop=True,
                            )
                            nc.vector.scalar_tensor_tensor(
                                out=a_t[:, hs : hs + 1024],
                                in0=m_tile[:, hs : hs + 1024],
                                scalar=NEG,
                                in1=qk,
                                op0=mybir.AluOpType.mult,
                                op1=mybir.AluOpType.add,
                            )

                        # p_unnorm = exp(a), rowsum via accum
                        p_un = pn_pool.tile([P, S], bf16, tag="p_un")
                        rsum = small_pool.tile([P, 1], f32, tag="rsum")
                        nc.scalar.activation(
                            p_un,
                            a_t,
                            mybir.ActivationFunctionType.Exp,
                            accum_out=rsum,
                        )
                        rinv = small_pool.tile([P, 1], f32, tag="rinv")
                        nc.vector.reciprocal(rinv, rsum)

                        p_out = pn_pool.tile([P, S], bf16, tag="p_out")
                        nc.vector.tensor_scalar_mul(p_out, p_un, rinv)
                        # SWDGE casts bf16 -> f32 during the store
                        nc.gpsimd.dma_start(out=outp[h, qs : qs + P, :], in_=p_out)

                        # transpose a into the corner-turn buffer (bf16)
                        for kq in range(2):
                            pt = ps_tr.tile([P, 1024], bf16, tag="ps_tr")
                            for kk in range(8):
                                kj = kq * 8 + kk
                                nc.tensor.transpose(
                                    pt[:, kk * P : (kk + 1) * P],
                                    a_t[:, kj * P : (kj + 1) * P],
                                    ident_bf,
                                )
                            dst = at_g[
                                :, kq * 8 : kq * 8 + 8, gq * P : (gq + 1) * P
                            ]
                            srcp = pt.rearrange("p (a b) -> p a b", a=8)
                            if kq == 1 and qi % 3 == 2:
                                nc.vector.tensor_copy(dst, srcp)
                            else:
                                nc.scalar.copy(dst, srcp)

                    # ---- AV for this group: attn_vT[64, GRP*P] ----
                    av = ps_av.tile([D, GRP * P], f32, tag="av")
                    for kj in range(KC):
                        nc.tensor.matmul(
                            av,
                            v_bf[:, kj, :],
                            at_g[:, kj, :],
                            start=(kj == 0),
                            stop=(kj == KC - 1),
                        )
                    av_sb = vo_pool.tile([D, GRP * P], bf16, tag="av_sb")
                    nc.vector.tensor_copy(av_sb, av)
                    # transpose back to [q, D] and write out
                    pt2 = ps_tr.tile([P, 1024], bf16, tag="ps_tr")
                    for gq in range(GRP):
                        nc.tensor.transpose(
                            pt2[:, gq * 256 : gq * 256 + D],
                            av_sb[:, gq * P : (gq + 1) * P],
                            ident_bf[:D, :D],
                        )
                    vo_sb = vo_pool.tile([P, GRP, D], f32, tag="vo_sb")
                    nc.vector.tensor_copy(
                        vo_sb,
                        pt2.rearrange("p (g c) -> p g c", g=4)[:, :, :D],
                    )
                    qs = g * GRP * P
                    nc.sync.dma_start(
                        out=outv[h, qs : qs + GRP * P, :].rearrange(
                            "(g p) d -> p g d", p=P
                        ),
                        in_=vo_sb,
                    )

    legalize_waits(nc)
    return nc


def kernel(Q, K, V, attn_mask):
    bf = ml_dtypes.bfloat16
    Q = np.ascontiguousarray(np.asarray(Q), dtype=np.float32)
    K = np.ascontiguousarray(np.asarray(K), dtype=np.float32)
    V = np.ascontiguousarray(np.asarray(V), dtype=np.float32)
    M = np.asarray(attn_mask).astype(np.uint8)

    # fold the 1/sqrt(D) scale into Q on the host; ship operands as bf16
    qf = (Q.reshape(B * H, S, D) * np.float32(SCALE)).astype(bf)
    kf = K.reshape(B * H, S, D).astype(bf)
    vf = V.reshape(B * H, S, D).astype(bf)
    mf = M.reshape(B * H, S, S)

    if "nc" not in _COMPILED:
        _COMPILED["nc"] = build_bass()
    nc = _COMPILED["nc"]

    in_maps = []
    for c in range(N_CORES):
        sl = slice(c * HEADS_PER_CORE, (c + 1) * HEADS_PER_CORE)
        in_maps.append(
            {
                "q": np.ascontiguousarray(qf[sl]),
                "k": np.ascontiguousarray(kf[sl]),
                "v": np.ascontiguousarray(vf[sl]),
                "m": np.ascontiguousarray(mf[sl]),
            }
        )

    res = run_bass_kernel_spmd(nc, in_maps, core_ids=list(range(N_CORES)))
    results = res.results

    attn_v = np.concatenate([results[c]["out_v"] for c in range(N_CORES)], axis=0)
    attn_p = np.concatenate([results[c]["out_p"] for c in range(N_CORES)], axis=0)
    attn_v = attn_v.reshape(B, H, S, D).astype(np.float32)
    attn_p = attn_p.reshape(B, H, S, S).astype(np.float32)
    return attn_v, attn_p


if __name__ == "__main__":
    rng = np.random.default_rng(0)
    Q = rng.standard_normal((B, H, S, D), dtype=np.float32)
    K = rng.standard_normal((B, H, S, D), dtype=np.float32)
    V = rng.standard_normal((B, H, S, D), dtype=np.float32)
    Mm = rng.integers(0, 2, size=(B, H, S, S)).astype(bool)
    av, ap = kernel(Q, K, V, Mm)
    print(av.shape, ap.shape, av.dtype, ap.dtype)


# revision 19
# speedup vs baseline: 6.3182x; 1.1309x over previous
"""Trainium2 Bass kernel for masked attention with pre-softmax-score AV matmul.

Reference semantics (faithful to the source module's bug):
    a = (Q @ K^T) / sqrt(D);  a = where(mask, -1e9, a)
    attn_p = softmax(a, axis=-1)
    attn_v = a @ V            # uses pre-softmax masked scores, NOT attn_p
    returns (attn_v, attn_p)

Shapes: Q,K,V (2,8,2048,64) f32; mask (2,8,2048,2048) bool.
Sharding: B*H = 16 head-slices, 2 per core across 8 cores (pure data
parallel, no collectives).
"""

import sys

sys.path.insert(0, "/opt/trn_rl_repo")

import ml_dtypes
import numpy as np

import concourse.bass as bass
import concourse.tile as tile
from concourse import mybir
from concourse.bass_utils import run_bass_kernel_spmd
from concourse.masks import make_identity

B, H, S, D = 2, 8, 2048, 64
N_CORES = 8
HEADS_PER_CORE = (B * H) // N_CORES  # 2
P = 128                              # partition tile (q rows per tile)
NQT = S // P                         # 16 q-tiles per head
KC = S // P                          # 16 k-chunks of 128
GRP = 4                              # q-tiles per AV matmul group
SCALE = 1.0 / np.sqrt(np.float32(D))  # 0.125, exact in fp32
NEG = -1.0e9

f32 = mybir.dt.float32
f32r = mybir.dt.float32r
bf16 = mybir.dt.bfloat16
u8 = mybir.dt.uint8

_COMPILED = {}


def legalize_waits(nc):
    """This walrus build allows only ONE sync-wait command per instruction.

    Tile's wait assigner can attach several (one per upstream engine), which
    fails codegen with 'Too many sync wait commands'. Hoist all but the last
    wait onto preceding same-engine NoOps (program order on the engine's
    sequencer makes this semantically identical).
    """
    n_fixed = 0
    for fn in nc.m.functions:
        for blk in fn.blocks:
            insts = list(blk.instructions)
            new_list = []
            changed = False
            for inst in insts:
                si = inst.sync_info
                if si is not None and si.on_wait is not None and len(si.on_wait) > 1:
                    waits = list(si.on_wait)
                    for wi, w in enumerate(waits[:-1]):
                        new_list.append(
                            mybir.InstNoOp(
                                name=f"{inst.name}-wn{wi}",
                                engine=inst.engine,
                                sync_info=mybir.SyncInfo(on_wait=[w], on_update=[]),
                            )
                        )
                    inst.sync_info = mybir.SyncInfo(
                        on_wait=waits[-1:], on_update=list(si.on_update or [])
                    )
                    changed = True
                    n_fixed += 1
                new_list.append(inst)
            if changed:
                blk.instructions = new_list
    return n_fixed


def build_bass():
    nc = bass.Bass()

    qt_ext = nc.declare_dram_parameter(
        "qt", [HEADS_PER_CORE, 2 * D, S], bf16, isOutput=False
    )
    kt_ext = nc.declare_dram_parameter(
        "kt", [HEADS_PER_CORE, 2 * D, S], bf16, isOutput=False
    )
    v_ext = nc.declare_dram_parameter("v", [HEADS_PER_CORE, S, D], bf16, isOutput=False)
    m_ext = nc.declare_dram_parameter("m", [HEADS_PER_CORE, S, S], u8, isOutput=False)
    outv = nc.declare_dram_parameter("out_v", [HEADS_PER_CORE, S, D], f32, isOutput=True)
    outp = nc.declare_dram_parameter("out_p", [HEADS_PER_CORE, S, S], f32, isOutput=True)

    with tile.TileContext(nc) as tc:
        with (
            tc.tile_pool(name="const", bufs=1) as const_pool,
            tc.tile_pool(name="head", bufs=2) as head_pool,
            tc.tile_pool(name="mask", bufs=8) as mask_pool,
            tc.tile_pool(name="sc", bufs=4) as sc_pool,
            tc.tile_pool(name="pn", bufs=4) as pn_pool,
            tc.tile_pool(name="at", bufs=3) as at_pool,
            tc.tile_pool(name="small", bufs=4) as small_pool,
            tc.tile_pool(name="vo", bufs=6) as vo_pool,
            tc.tile_pool(name="ps_qk", bufs=2, space="PSUM") as ps_qk,
            tc.tile_pool(name="ps_tr", bufs=2, space="PSUM") as ps_tr,
            tc.tile_pool(name="ps_av", bufs=2, space="PSUM") as ps_av,
        ):
            ident = const_pool.tile([P, P], f32)
            make_identity(nc, ident)
            ident_bf = const_pool.tile([P, P], bf16)
            nc.vector.tensor_copy(ident_bf, ident)

            for h in range(HEADS_PER_CORE):
                # Q^T (scaled) and K^T arrive pre-transposed and duplicated
                # across both partition halves (built on host) so QK matmuls
                # can row-pack without any on-chip prep.
                qt = head_pool.tile([2 * D, S], bf16, tag="qt")
                kt = head_pool.tile([2 * D, S], bf16, tag="kt")
                v_bf = head_pool.tile([P, KC, D], bf16, tag="v_bf")
                nc.sync.dma_start(out=qt, in_=qt_ext[h])
                nc.sync.dma_start(out=kt, in_=kt_ext[h])
                nc.sync.dma_start(
                    out=v_bf, in_=v_ext[h].rearrange("(t p) d -> p t d", p=P)
                )

                for g in range(NQT // GRP):
                    # corner-turn buffer: aT_g[p, kc, q_in_group]
                    at_g = at_pool.tile([P, KC, GRP * P], bf16, tag="at_g")

                    for gq in range(GRP):
                        qi = g * GRP + gq
                        qs = qi * P

                        m_tile = mask_pool.tile([P, S], u8, tag="m")
                        nc.sync.dma_start(out=m_tile, in_=m_ext[h, qs : qs + P, :])

                        # a = qk + mask * (-1e9)  (masked scores, bf16),
                        # in two pipelined halves so PE can start the next
                        # half/tile while DVE drains the previous one
                        a_t = sc_pool.tile([P, S], bf16, tag="a")
                        for half in range(2):
                            hs = half * 1024
                            qk = ps_qk.tile([P, 1024], f32, tag="qk")
                            nc.tensor.matmul(
                                qk[:, 0:512],
                                qt[:D, qs : qs + P],
                                kt[:D, hs : hs + 512],
                                start=True,
                                stop=True,
                            )
                            nc.tensor.matmul(
                                qk[:, 512:1024],
                                qt[D:, qs : qs + P],
                                kt[D:, hs + 512 : hs + 1024],
                                start=True,
                                stop=True,
                            )
                            nc.vector.scalar_tensor_tensor(
                                out=a_t[:, hs : hs + 1024],
                                in0=m_tile[:, hs : hs + 1024],
                                scalar=NEG,
                                in1=qk,
                                op0=mybir.AluOpType.mult,
                                op1=mybir.AluOpType.add,
                            )

                        # p_unnorm = exp(a), rowsum via accum
                        p_un = pn_pool.tile([P, S], bf16, tag="p_un")
                        rsum = small_pool.tile([P, 1], f32, tag="rsum")
                        nc.scalar.activation(
                            p_un,
                            a_t,
                            mybir.ActivationFunctionType.Exp,
                            accum_out=rsum,
                        )
                        rinv = small_pool.tile([P, 1], f32, tag="rinv")
                        nc.vector.reciprocal(rinv, rsum)

                        p_out = pn_pool.tile([P, S], bf16, tag="p_out")
                        nc.vector.tensor_scalar_mul(p_out, p_un, rinv)
                        # SWDGE casts bf16 -> f32 during the store
                        nc.gpsimd.dma_start(out=outp[h, qs : qs + P, :], in_=p_out)

                        # transpose a into the corner-turn buffer (bf16)
                        for kq in range(2):
                            pt = ps_tr.tile([P, 1024], bf16, tag="ps_tr")
                            for kk in range(8):
                                kj = kq * 8 + kk
                                nc.tensor.transpose(
                                    pt[:, kk * P : (kk + 1) * P],
                                    a_t[:, kj * P : (kj + 1) * P],
                                    ident_bf,
                                )
                            dst = at_g[
                                :, kq * 8 : kq * 8 + 8, gq * P : (gq + 1) * P
                            ]
                            srcp = pt.rearrange("p (a b) -> p a b", a=8)
                            if kq == 1 and qi % 3 == 2:
                                nc.vector.tensor_copy(dst, srcp)
                            else:
                                nc.scalar.copy(dst, srcp)

                    # ---- AV for this group: attn_vT[64, GRP*P] ----
                    av = ps_av.tile([D, GRP * P], f32, tag="av")
                    for kj in range(KC):
                        nc.tensor.matmul(
                            av,
                            v_bf[:, kj, :],
                            at_g[:, kj, :],
                            start=(kj == 0),
                            stop=(kj == KC - 1),
                        )
                    av_sb = vo_pool.tile([D, GRP * P], bf16, tag="av_sb")
                    nc.vector.tensor_copy(av_sb, av)
                    # transpose back to [q, D] and write out
                    pt2 = ps_tr.tile([P, 1024], bf16, tag="ps_tr")
                    for gq in range(GRP):
                        nc.tensor.transpose(
                            pt2[:, gq * 256 : gq * 256 + D],
                            av_sb[:, gq * P : (gq + 1) * P],
                            ident_bf[:D, :D],
                        )
                    vo_sb = vo_pool.tile([P, GRP, D], f32, tag="vo_sb")
                    nc.vector.tensor_copy(
                        vo_sb,
                        pt2.rearrange("p (g c) -> p g c", g=4)[:, :, :D],
                    )
                    qs = g * GRP * P
                    nc.sync.dma_start(
                        out=outv[h, qs : qs + GRP * P, :].rearrange(
                            "(g p) d -> p g d", p=P
                        ),
                        in_=vo_sb,
                    )

    legalize_waits(nc)
    return nc


def kernel(Q, K, V, attn_mask):
    bf = ml_dtypes.bfloat16
    Q = np.ascontiguousarray(np.asarray(Q), dtype=np.float32)
    K = np.ascontiguousarray(np.asarray(K), dtype=np.float32)
    V = np.ascontiguousarray(np.asarray(V), dtype=np.float32)
    M = np.asarray(attn_mask).astype(np.uint8)

    # fold the 1/sqrt(D) scale into Q on the host; ship Q^T/K^T bf16,
    # duplicated across both partition halves for row-packed QK matmuls
    qT = np.ascontiguousarray(
        (Q.reshape(B * H, S, D) * np.float32(SCALE)).astype(bf).transpose(0, 2, 1)
    )
    kT = np.ascontiguousarray(K.reshape(B * H, S, D).astype(bf).transpose(0, 2, 1))
    qf = np.concatenate([qT, qT], axis=1)
    kf = np.concatenate([kT, kT], axis=1)
    vf = V.reshape(B * H, S, D).astype(bf)
    mf = M.reshape(B * H, S, S)

    if "nc" not in _COMPILED:
        _COMPILED["nc"] = build_bass()
    nc = _COMPILED["nc"]

    in_maps = []
    for c in range(N_CORES):
        sl = slice(c * HEADS_PER_CORE, (c + 1) * HEADS_PER_CORE)
        in_maps.append(
            {
                "qt": np.ascontiguousarray(qf[sl]),
                "kt": np.ascontiguousarray(kf[sl]),
                "v": np.ascontiguousarray(vf[sl]),
                "m": np.ascontiguousarray(mf[sl]),
            }
        )

    res = run_bass_kernel_spmd(nc, in_maps, core_ids=list(range(N_CORES)))
    results = res.results

    attn_v = np.concatenate([results[c]["out_v"] for c in range(N_CORES)], axis=0)
    attn_p = np.concatenate([results[c]["out_p"] for c in range(N_CORES)], axis=0)
    attn_v = attn_v.reshape(B, H, S, D).astype(np.float32)
    attn_p = attn_p.reshape(B, H, S, S).astype(np.float32)
    return attn_v, attn_p


if __name__ == "__main__":
    rng = np.random.default_rng(0)
    Q = rng.standard_normal((B, H, S, D), dtype=np.float32)
    K = rng.standard_normal((B, H, S, D), dtype=np.float32)
    V = rng.standard_normal((B, H, S, D), dtype=np.float32)
    Mm = rng.integers(0, 2, size=(B, H, S, S)).astype(bool)
    av, ap = kernel(Q, K, V, Mm)
    print(av.shape, ap.shape, av.dtype, ap.dtype)


# revision 20
# speedup vs baseline: 6.3748x; 1.0090x over previous
"""Trainium2 Bass kernel for masked attention with pre-softmax-score AV matmul.

Reference semantics (faithful to the source module's bug):
    a = (Q @ K^T) / sqrt(D);  a = where(mask, -1e9, a)
    attn_p = softmax(a, axis=-1)
    attn_v = a @ V            # uses pre-softmax masked scores, NOT attn_p
    returns (attn_v, attn_p)

Shapes: Q,K,V (2,8,2048,64) f32; mask (2,8,2048,2048) bool.
Sharding: B*H = 16 head-slices, 2 per core across 8 cores (pure data
parallel, no collectives).
"""

import sys

sys.path.insert(0, "/opt/trn_rl_repo")

import ml_dtypes
import numpy as np

import concourse.bass as bass
import concourse.tile as tile
from concourse import mybir
from concourse.bass_utils import run_bass_kernel_spmd
from concourse.masks import make_identity

B, H, S, D = 2, 8, 2048, 64
N_CORES = 8
HEADS_PER_CORE = (B * H) // N_CORES  # 2
P = 128                              # partition tile (q rows per tile)
NQT = S // P                         # 16 q-tiles per head
KC = S // P                          # 16 k-chunks of 128
GRP = 4                              # q-tiles per AV matmul group
SCALE = 1.0 / np.sqrt(np.float32(D))  # 0.125, exact in fp32
NEG = -1.0e9

f32 = mybir.dt.float32
f32r = mybir.dt.float32r
bf16 = mybir.dt.bfloat16
u8 = mybir.dt.uint8

_COMPILED = {}


def legalize_waits(nc):
    """This walrus build allows only ONE sync-wait command per instruction.

    Tile's wait assigner can attach several (one per upstream engine), which
    fails codegen with 'Too many sync wait commands'. Hoist all but the last
    wait onto preceding same-engine NoOps (program order on the engine's
    sequencer makes this semantically identical).
    """
    n_fixed = 0
    for fn in nc.m.functions:
        for blk in fn.blocks:
            insts = list(blk.instructions)
            new_list = []
            changed = False
            for inst in insts:
                si = inst.sync_info
                if si is not None and si.on_wait is not None and len(si.on_wait) > 1:
                    waits = list(si.on_wait)
                    for wi, w in enumerate(waits[:-1]):
                        new_list.append(
                            mybir.InstNoOp(
                                name=f"{inst.name}-wn{wi}",
                                engine=inst.engine,
                                sync_info=mybir.SyncInfo(on_wait=[w], on_update=[]),
                            )
                        )
                    inst.sync_info = mybir.SyncInfo(
                        on_wait=waits[-1:], on_update=list(si.on_update or [])
                    )
                    changed = True
                    n_fixed += 1
                new_list.append(inst)
            if changed:
                blk.instructions = new_list
    return n_fixed


def build_bass():
    nc = bass.Bass()

    qt_ext = nc.declare_dram_parameter(
        "qt", [HEADS_PER_CORE, 2 * D, S], bf16, isOutput=False
    )
    kt_ext = nc.declare_dram_parameter(
        "kt", [HEADS_PER_CORE, 2 * D, S], bf16, isOutput=False
    )
    v_ext = nc.declare_dram_parameter("v", [HEADS_PER_CORE, S, D], bf16, isOutput=False)
    m_ext = nc.declare_dram_parameter("m", [HEADS_PER_CORE, S, S], u8, isOutput=False)
    outv = nc.declare_dram_parameter("out_v", [HEADS_PER_CORE, S, D], f32, isOutput=True)
    outp = nc.declare_dram_parameter("out_p", [HEADS_PER_CORE, S, S], f32, isOutput=True)

    with tile.TileContext(nc) as tc:
        with (
            tc.tile_pool(name="const", bufs=1) as const_pool,
            tc.tile_pool(name="head", bufs=2) as head_pool,
            tc.tile_pool(name="mask", bufs=8) as mask_pool,
            tc.tile_pool(name="sc", bufs=4) as sc_pool,
            tc.tile_pool(name="pn", bufs=4) as pn_pool,
            tc.tile_pool(name="at", bufs=3) as at_pool,
            tc.tile_pool(name="small", bufs=4) as small_pool,
            tc.tile_pool(name="vo", bufs=6) as vo_pool,
            tc.tile_pool(name="ps_qk", bufs=2, space="PSUM") as ps_qk,
            tc.tile_pool(name="ps_tr", bufs=2, space="PSUM") as ps_tr,
            tc.tile_pool(name="ps_av", bufs=2, space="PSUM") as ps_av,
        ):
            ident = const_pool.tile([P, P], f32)
            make_identity(nc, ident)
            ident_bf = const_pool.tile([P, P], bf16)
            nc.vector.tensor_copy(ident_bf, ident)

            for h in range(HEADS_PER_CORE):
                # Q^T (scaled) and K^T arrive pre-transposed and duplicated
                # across both partition halves (built on host) so QK matmuls
                # can row-pack without any on-chip prep.
                qt = head_pool.tile([2 * D, S], bf16, tag="qt")
                kt = head_pool.tile([2 * D, S], bf16, tag="kt")
                v_bf = head_pool.tile([P, KC, D], bf16, tag="v_bf")
                nc.sync.dma_start(out=qt, in_=qt_ext[h])
                nc.sync.dma_start(out=kt, in_=kt_ext[h])
                nc.sync.dma_start(
                    out=v_bf, in_=v_ext[h].rearrange("(t p) d -> p t d", p=P)
                )

                for g in range(NQT // GRP):
                    # corner-turn buffer: aT_g[p, kc, q_in_group]
                    at_g = at_pool.tile([P, KC, GRP * P], bf16, tag="at_g")

                    for gq in range(GRP):
                        qi = g * GRP + gq
                        qs = qi * P

                        m_tile = mask_pool.tile([P, S], u8, tag="m")
                        nc.sync.dma_start(out=m_tile, in_=m_ext[h, qs : qs + P, :])

                        # a = qk + mask * (-1e9)  (masked scores, bf16),
                        # in two pipelined halves so PE can start the next
                        # half/tile while DVE drains the previous one
                        a_t = sc_pool.tile([P, S], bf16, tag="a")
                        for half in range(2):
                            hs = half * 1024
                            qk = ps_qk.tile([P, 1024], f32, tag="qk")
                            nc.tensor.matmul(
                                qk[:, 0:512],
                                qt[:D, qs : qs + P],
                                kt[:D, hs : hs + 512],
                                start=True,
                                stop=True,
                            )
                            nc.tensor.matmul(
                                qk[:, 512:1024],
                                qt[D:, qs : qs + P],
                                kt[D:, hs + 512 : hs + 1024],
                                start=True,
                                stop=True,
                            )
                            nc.vector.scalar_tensor_tensor(
                                out=a_t[:, hs : hs + 1024],
                                in0=m_tile[:, hs : hs + 1024],
                                scalar=NEG,
                                in1=qk,
                                op0=mybir.AluOpType.mult,
                                op1=mybir.AluOpType.add,
                            )

                        # p_unnorm = exp(a), rowsum via accum
                        p_un = pn_pool.tile([P, S], bf16, tag="p_un")
                        rsum = small_pool.tile([P, 1], f32, tag="rsum")
                        nc.scalar.activation(
                            p_un,
                            a_t,
                            mybir.ActivationFunctionType.Exp,
                            accum_out=rsum,
                        )
                        rinv = small_pool.tile([P, 1], f32, tag="rinv")
                        nc.vector.reciprocal(rinv, rsum)

                        p_out = pn_pool.tile([P, S], bf16, tag="p_out")
                        nc.vector.tensor_scalar_mul(p_out, p_un, rinv)
                        # SWDGE casts bf16 -> f32 during the store
                        nc.gpsimd.dma_start(out=outp[h, qs : qs + P, :], in_=p_out)

                        # transpose a into the corner-turn buffer (bf16)
                        for kq in range(2):
                            pt = ps_tr.tile([P, 1024], bf16, tag="ps_tr")
                            for kk in range(8):
                                kj = kq * 8 + kk
                                nc.tensor.transpose(
                                    pt[:, kk * P : (kk + 1) * P],
                                    a_t[:, kj * P : (kj + 1) * P],
                                    ident_bf,
                                )
                            dst = at_g[
                                :, kq * 8 : kq * 8 + 8, gq * P : (gq + 1) * P
                            ]
                            srcp = pt.rearrange("p (a b) -> p a b", a=8)
                            if kq == 1 and qi % 2 == 1:
                                nc.vector.tensor_copy(dst, srcp)
                            else:
                                nc.scalar.copy(dst, srcp)

                    # ---- AV for this group: attn_vT[64, GRP*P] ----
                    av = ps_av.tile([D, GRP * P], f32, tag="av")
                    for kj in range(KC):
                        nc.tensor.matmul(
                            av,
                            v_bf[:, kj, :],
                            at_g[:, kj, :],
                            start=(kj == 0),
                            stop=(kj == KC - 1),
                        )
                    av_sb = vo_pool.tile([D, GRP * P], bf16, tag="av_sb")
                    nc.vector.tensor_copy(av_sb, av)
                    # transpose back to [q, D] and write out
                    pt2 = ps_tr.tile([P, 1024], bf16, tag="ps_tr")
                    for gq in range(GRP):
                        nc.tensor.transpose(
                            pt2[:, gq * 256 : gq * 256 + D],
                            av_sb[:, gq * P : (gq + 1) * P],
                            ident_bf[:D, :D],
                        )
                    vo_sb = vo_pool.tile([P, GRP, D], f32, tag="vo_sb")
                    nc.vector.tensor_copy(
                        vo_sb,
                        pt2.rearrange("p (g c) -> p g c", g=4)[:, :, :D],
                    )
                    qs = g * GRP * P
                    nc.sync.dma_start(
                        out=outv[h, qs : qs + GRP * P, :].rearrange(
                            "(g p) d -> p g d", p=P
                        ),
                        in_=vo_sb,
                    )

    legalize_waits(nc)
    return nc


def kernel(Q, K, V, attn_mask):
    bf = ml_dtypes.bfloat16
    Q = np.ascontiguousarray(np.asarray(Q), dtype=np.float32)
    K = np.ascontiguousarray(np.asarray(K), dtype=np.float32)
    V = np.ascontiguousarray(np.asarray(V), dtype=np.float32)
    M = np.asarray(attn_mask).astype(np.uint8)

    # fold the 1/sqrt(D) scale into Q on the host; ship Q^T/K^T bf16,
    # duplicated across both partition halves for row-packed QK matmuls
    qT = np.ascontiguousarray(
        (Q.reshape(B * H, S, D) * np.float32(SCALE)).astype(bf).transpose(0, 2, 1)
    )
    kT = np.ascontiguousarray(K.reshape(B * H, S, D).astype(bf).transpose(0, 2, 1))
    qf = np.concatenate([qT, qT], axis=1)
    kf = np.concatenate([kT, kT], axis=1)
    vf = V.reshape(B * H, S, D).astype(bf)
    mf = M.reshape(B * H, S, S)

    if "nc" not in _COMPILED:
        _COMPILED["nc"] = build_bass()
    nc = _COMPILED["nc"]

    in_maps = []
    for c in range(N_CORES):
        sl = slice(c * HEADS_PER_CORE, (c + 1) * HEADS_PER_CORE)
        in_maps.append(
            {
                "qt": np.ascontiguousarray(qf[sl]),
                "kt": np.ascontiguousarray(kf[sl]),
                "v": np.ascontiguousarray(vf[sl]),
                "m": np.ascontiguousarray(mf[sl]),
            }
        )

    res = run_bass_kernel_spmd(nc, in_maps, core_ids=list(range(N_CORES)))
    results = res.results

    attn_v = np.concatenate([results[c]["out_v"] for c in range(N_CORES)], axis=0)
    attn_p = np.concatenate([results[c]["out_p"] for c in range(N_CORES)], axis=0)
    attn_v = attn_v.reshape(B, H, S, D).astype(np.float32)
    attn_p = attn_p.reshape(B, H, S, S).astype(np.float32)
    return attn_v, attn_p


if __name__ == "__main__":
    rng = np.random.default_rng(0)
    Q = rng.standard_normal((B, H, S, D), dtype=np.float32)
    K = rng.standard_normal((B, H, S, D), dtype=np.float32)
    V = rng.standard_normal((B, H, S, D), dtype=np.float32)
    Mm = rng.integers(0, 2, size=(B, H, S, S)).astype(bool)
    av, ap = kernel(Q, K, V, Mm)
    print(av.shape, ap.shape, av.dtype, ap.dtype)
